# revision 10
# baseline (speedup 1.0000x reference)
"""Trainium2 Bass kernel for a GPT-style decoder block (B=2, T=2048, C=768, H=12).

Sharding: 8 cores = 2 batches x 4 interleaved block-sets. Core c owns 128-row
blocks {c, c+4, c+8, c+12} of its batch. Its context buffer holds the 16
position-blocks [zeros x (3-c) | blocks 0..12+c]; the own blocks then sit at
the STATIC positions {3, 7, 11, 15} with causal context = position prefixes of
length {4, 8, 12, 16} blocks. Every core therefore runs the same instruction
stream while doing the load-balanced share (40/64) of the causal attention
work; the inserted zero blocks are masked via a per-block validity scale on V
(and on the denominator ones-column), so they contribute exactly 0 to both the
attention numerator and the softmax denominator.

Numerics: Q/K/V projections and the second MLP matmul run in fp8e4 with
DoubleRow perf mode (two 128-channel contraction chunks per instruction);
scores, P, V and the first MLP matmul stay bf16 (fp8 everywhere pushes the
fixed-seed rel-err past the 2e-2 gate; this mix measures ~1.6e-2 in numpy).
LN statistics, softmax normalization, residuals and the output stay fp32.
fp8 scales (weights x512, activations x16) are divided out on PSUM->SBUF
copies.

Note: reference computes scores = K @ Q^T, so the output-row operand is K (own
rows) and the context operand is Q/V. The causal triangle on each own block's
diagonal position is applied by accumulating a -1e5 upper-triangle mask into
the scores PSUM with one extra bf16 matmul. V / Q projections are interleaved
into the LN1 loop so TensorE stays busy during the per-tile LN chains.
"""

import os

import numpy as np

B, T, C = 2, 2048, 768
H, DH = 12, 64
F = 4 * C
R = 512            # own rows per core
NT = 16            # ctx position blocks
NC = C // 128      # 6
JC = NC // 2       # 3 channel pairs
NF = F // 128      # 24
JF = NF // 2       # 12 hidden pairs
HP = H // 2        # 6 head pairs
VS = 66            # per-head stride in Vt (64 v + 1 ones + pad)
EPS = 1e-3
SX = 16.0          # fp8 scale on normalized activations
SW = 512.0         # fp8 scale on weights
SXW = SX * SW      # 8192

_CACHE = {}


def _build_program():
    import concourse.bass as bass  # noqa: F401
    import concourse.mybir as mybir
    import concourse.tile as tile
    from concourse import bacc

    dt = mybir.dt
    f32 = dt.float32
    bf16 = dt.bfloat16
    fp8 = dt.float8e4
    AF = mybir.ActivationFunctionType
    ALU = mybir.AluOpType
    PM = mybir.MatmulPerfMode

    nc = bacc.Bacc("TRN2", target_bir_lowering=False, debug=False, num_devices=8)

    # ---- DRAM I/O ----
    x_ctx = nc.dram_tensor("x_ctx", [T, C], f32, kind="ExternalInput")
    validv_d = nc.dram_tensor("validv", [128, NT], f32, kind="ExternalInput")
    vones_d = nc.dram_tensor("vones", [128, NT], f32, kind="ExternalInput")
    wq_d = nc.dram_tensor("wq8", [128, JC, 2, C], fp8, kind="ExternalInput")
    wk_d = nc.dram_tensor("wk8", [128, JC, 2, C], fp8, kind="ExternalInput")
    wv_d = nc.dram_tensor("wv8", [128, JC, 2, C], fp8, kind="ExternalInput")
    w1_d = nc.dram_tensor("w1b", [128, NC, F], bf16, kind="ExternalInput")
    w2_d = nc.dram_tensor("w28", [128, JF, 2, C], fp8, kind="ExternalInput")
    bq_d = nc.dram_tensor("bq", [128, HP], f32, kind="ExternalInput")
    bk_d = nc.dram_tensor("bk", [128, HP], f32, kind="ExternalInput")
    b1_d = nc.dram_tensor("b1", [128, NF], f32, kind="ExternalInput")
    b2_d = nc.dram_tensor("b2row", [1, C], bf16, kind="ExternalInput")
    g1b_d = nc.dram_tensor("g1b", [128, C], bf16, kind="ExternalInput")
    b1rb_d = nc.dram_tensor("b1rb", [128, C], f32, kind="ExternalInput")
    tri_d = nc.dram_tensor("trimask", [128, 128], bf16, kind="ExternalInput")
    ident_d = nc.dram_tensor("identb", [128, 128], bf16, kind="ExternalInput")
    out_d = nc.dram_tensor("out", [R, C], f32, kind="ExternalOutput")

    OWN = (3, 7, 11, 15)  # own position blocks (slot s -> position 4s+3)

    with tile.TileContext(nc) as tc:
        with (
            tc.tile_pool(name="const", bufs=1) as constp,
            tc.tile_pool(name="keep", bufs=1) as keepp,
            tc.tile_pool(name="w2pool", bufs=1) as w2p,
        ):
            validv = constp.tile([128, NT], f32)
            nc.sync.dma_start(validv[:], validv_d[:])
            vones = constp.tile([128, NT], f32)
            nc.sync.dma_start(vones[:], vones_d[:])
            tri = constp.tile([128, 128], bf16)
            nc.sync.dma_start(tri[:], tri_d[:])
            ident = constp.tile([128, 128], bf16)
            nc.sync.dma_start(ident[:], ident_d[:])
            bqs = constp.tile([128, HP], f32)
            nc.sync.dma_start(bqs[:], bq_d[:])
            bks = constp.tile([128, HP], f32)
            nc.sync.dma_start(bks[:], bk_d[:])
            b1s = constp.tile([128, NF], f32)
            nc.sync.dma_start(b1s[:], b1_d[:])
            b2row = constp.tile([1, C], bf16)
            nc.sync.dma_start(b2row[:], b2_d[:])
            g1b = constp.tile([128, C], bf16)
            nc.sync.dma_start(g1b[:], g1b_d[:])
            b1rb = constp.tile([128, C], f32)
            nc.sync.dma_start(b1rb[:], b1rb_d[:])
            ones1 = constp.tile([1, 128], bf16)
            nc.vector.memset(ones1[:], 1.0)
            onesf = constp.tile([128, H, 1], f32)
            nc.vector.memset(onesf[:], 1.0)
            eps_t = constp.tile([128, 1], f32)
            nc.vector.memset(eps_t[:], EPS)

            # w2 (fp8, small) arrives early so its DMA overlaps everything
            w2 = w2p.tile([128, JF, 2, C], fp8, name="w28")
            nc.sync.dma_start(w2[:], w2_d[:])

            xn_keep = keepp.tile([128, 4, C], f32)   # own rows (slot order), fp32
            x1 = keepp.tile([128, 4, C], f32)        # post-attention residual
            y_sb = keepp.tile([128, 4, H, 65], bf16)  # y token-major; k=3-s order

            with (
                tc.tile_pool(name="xnT8", bufs=1) as xnT8p,
                tc.tile_pool(name="QT", bufs=1) as QTp,
                tc.tile_pool(name="KT", bufs=1) as KTp,
                tc.tile_pool(name="V", bufs=1) as Vp,
            ):
                xnT8 = xnT8p.tile([128, JC, 2, T], fp8)       # xn^T * 16
                QT = QTp.tile([128, HP, T], bf16)             # q (true scale)
                KT = KTp.tile([128, HP, R], bf16)             # k own, col k=3-s
                xnT8own = KTp.tile([128, JC, 2, R], fp8)
                Vt = Vp.tile([128, NT, H, VS], bf16)          # v true, [..,64]=1

                # ===== Phase A+B: LN1, transpose, QKV (interleaved) =====
                with (
                    tc.tile_pool(name="xin", bufs=3) as xinp,
                    tc.tile_pool(name="stat", bufs=4) as statp,
                    tc.tile_pool(name="xnbf", bufs=3) as xnbfp,
                    tc.tile_pool(name="wqkv", bufs=1) as wp,
                    tc.tile_pool(name="psT", bufs=2, space="PSUM") as psT,
                    tc.tile_pool(name="psQ", bufs=2, space="PSUM") as psQ,
                    tc.tile_pool(name="psV", bufs=2, space="PSUM") as psV,
                ):
                    wq = wp.tile([128, JC, 2, C], fp8, name="wq8")
                    nc.sync.dma_start(wq[:], wq_d[:])
                    wk = wp.tile([128, JC, 2, C], fp8, name="wk8")
                    nc.sync.dma_start(wk[:], wk_d[:])
                    wv = wp.tile([128, JC, 2, C], fp8, name="wv8")
                    nc.sync.dma_start(wv[:], wv_d[:])

                    TILE_ORDER = (3, 7, 11, 15, 0, 1, 2, 4, 5, 6,
                                  8, 9, 10, 12, 13, 14)
                    qdone = [False] * 4
                    done = set()

                    def emit_front(tb):
                        # DMA + LN stats + normalize (vector/ACT, no PSUM)
                        xt = xinp.tile([128, C], f32, tag="xt", name="xt")
                        nc.sync.dma_start(xt[:], x_ctx[tb * 128:(tb + 1) * 128, :])
                        st6 = statp.tile([128, 2, 6], f32, tag="st6", name="st6")
                        for g in range(2):
                            nc.vector.bn_stats(
                                st6[:, g, :], xt[:, g * 384:(g + 1) * 384]
                            )
                        st2 = statp.tile([128, 2], f32, tag="st2", name="st2")
                        nc.vector.bn_aggr(st2[:], st6[:])
                        std = statp.tile([128, 1], f32, tag="std", name="std")
                        nc.scalar.activation(std[:], st2[:, 1:2], AF.Sqrt, bias=eps_t[:])
                        rstd = statp.tile([128, 1], f32, tag="rstd", name="rstd")
                        nc.vector.reciprocal(rstd[:], std[:])
                        nmb = statp.tile([128, 1], f32, tag="nmb", name="nmb")
                        nc.vector.tensor_scalar(
                            nmb[:], st2[:, 0:1], rstd[:], -1.0,
                            op0=ALU.mult, op1=ALU.mult,
                        )
                        xn_bf = xnbfp.tile([128, C], bf16, tag="xn_bf", name="xn_bf")
                        nc.scalar.activation(
                            xn_bf[:], xt[:], AF.Identity, bias=nmb[:], scale=rstd[:]
                        )
                        if tb in OWN:
                            s = OWN.index(tb)
                            nc.gpsimd.tensor_scalar(
                                xn_keep[:, s, :], xt[:], st2[:, 0:1], rstd[:],
                                op0=ALU.subtract, op1=ALU.mult,
                            )
                        return xn_bf

                    def emit_back(tb, xn_bf):
                        done.add(tb)
                        tp = psT.tile([128, JC, 2, 128], bf16, tag="psT", name="tp")
                        for cb in range(NC):
                            nc.tensor.matmul(
                                tp[:, cb // 2, cb % 2, :],
                                xn_bf[:, cb * 128:(cb + 1) * 128],
                                ident[:], is_transpose=True, start=True, stop=True,
                            )
                        if tb % 2 == 0:
                            nc.vector.tensor_scalar(
                                xnT8[:, :, :, tb * 128:(tb + 1) * 128], tp[:],
                                SX, None, op0=ALU.mult,
                            )
                        else:
                            nc.scalar.mul(
                                xnT8[:, :, :, tb * 128:(tb + 1) * 128], tp[:], SX
                            )

                        # V projection for this tile (fp8 DoubleRow)
                        for g in range(2):
                            ps = psV.tile([128, 6, 64], f32, tag="psV", name="psv")
                            for j in range(JC):
                                nc.tensor.matmul(
                                    ps[:], xnT8[:, j, :, tb * 128:(tb + 1) * 128],
                                    wv[:, j, :, g * 384:(g + 1) * 384],
                                    start=(j == 0), stop=(j == JC - 1),
                                    perf_mode=PM.DoubleRow,
                                )
                            if (tb + g) % 2 == 0:
                                nc.vector.tensor_scalar(
                                    Vt[:, tb, g * 6:(g + 1) * 6, 0:64],
                                    ps[:], validv[:, tb:tb + 1], None, op0=ALU.mult,
                                )
                            else:
                                nc.scalar.activation(
                                    Vt[:, tb, g * 6:(g + 1) * 6, 0:64], ps[:],
                                    AF.Identity, scale=validv[:, tb:tb + 1],
                                )
                        nc.vector.tensor_scalar(
                            Vt[:, tb, :, 64:65], onesf[:],
                            vones[:, tb:tb + 1], None, op0=ALU.mult,
                        )
                        if tb in OWN:
                            s = OWN.index(tb)
                            nc.gpsimd.tensor_copy(
                                xnT8own[:, :, :, (3 - s) * 128:(4 - s) * 128],
                                xnT8[:, :, :, tb * 128:(tb + 1) * 128],
                            )
                        if all(p in done for p in OWN) and not qdone[0] \
                                and tb == 15:
                            qdone[0] = True
                            for hp in range(HP):
                                ps = psQ.tile([128, 512], f32, tag="psQ", name="psk")
                                for j in range(JC):
                                    nc.tensor.matmul(
                                        ps[:], wk[:, j, :, hp * 128:(hp + 1) * 128],
                                        xnT8own[:, j, :, :],
                                        start=(j == 0), stop=(j == JC - 1),
                                        perf_mode=PM.DoubleRow,
                                    )
                                nc.vector.tensor_scalar(
                                    KT[:, hp, :], ps[:], 1.0 / SXW, bks[:, hp:hp + 1],
                                    op0=ALU.mult, op1=ALU.add,
                                )
                        for nb in range(4):
                            grp = {4 * nb, 4 * nb + 1, 4 * nb + 2, 4 * nb + 3}
                            key = "q%d" % nb
                            if not (grp <= done) or key in done:
                                continue
                            done.add(key)
                            for hp in range(HP):
                                ps = psQ.tile([128, 512], f32, tag="psQ", name="psq")
                                for j in range(JC):
                                    nc.tensor.matmul(
                                        ps[:], wq[:, j, :, hp * 128:(hp + 1) * 128],
                                        xnT8[:, j, :, nb * 512:(nb + 1) * 512],
                                        start=(j == 0), stop=(j == JC - 1),
                                        perf_mode=PM.DoubleRow,
                                    )
                                if hp % 2 == 0:
                                    nc.scalar.activation(
                                        QT[:, hp, nb * 512:(nb + 1) * 512], ps[:],
                                        AF.Identity, bias=bqs[:, hp:hp + 1],
                                        scale=1.0 / SXW,
                                    )
                                else:
                                    nc.vector.tensor_scalar(
                                        QT[:, hp, nb * 512:(nb + 1) * 512], ps[:],
                                        1.0 / SXW, bqs[:, hp:hp + 1],
                                        op0=ALU.mult, op1=ALU.add,
                                    )

                    # pipeline: front(t+1) emitted before back(t)
                    pend = None
                    for tb in TILE_ORDER:
                        nf_ = emit_front(tb)
                        if pend is not None:
                            emit_back(*pend)
                        pend = (tb, nf_)
                    emit_back(*pend)

                # ===== Phase C: attention =====
                with (
                    tc.tile_pool(name="exps", bufs=2) as expp,
                    tc.tile_pool(name="yT", bufs=2) as ytp,
                    tc.tile_pool(name="psS", bufs=2, space="PSUM") as psS,
                    tc.tile_pool(name="psY", bufs=2, space="PSUM") as psY,
                    tc.tile_pool(name="psTy", bufs=2, space="PSUM") as psTy,
                ):
                    def emit_scores(h, expST):
                        # generator: one step per ctx pair (scores + exp)
                        hp, off = h // 2, 64 * (h % 2)
                        for jp in range(NT // 2):
                            Np = (4 - jp // 2) * 128
                            ps = psS.tile([128, 2, 512], f32, tag="psS", name="pss")
                            for ql in range(2):
                                P = 2 * jp + ql
                                diag = (P % 4 == 3)
                                nc.tensor.matmul(
                                    ps[:, ql, 0:Np],
                                    QT[off:off + 64, hp, P * 128:(P + 1) * 128],
                                    KT[off:off + 64, hp, 0:Np],
                                    start=True, stop=not diag,
                                    skip_group_check=diag,
                                )
                                if diag:
                                    nc.tensor.matmul(
                                        ps[:, ql, Np - 128:Np],
                                        ident[:], tri[:],
                                        start=False, stop=True,
                                        skip_group_check=True,
                                    )
                            nc.scalar.activation(
                                expST[:, 2 * jp:2 * jp + 2, 0:Np], ps[:, :, 0:Np],
                                AF.Exp, scale=0.125,
                            )
                            yield

                    def emit_pv(h, expST):
                        # generator: one step per ctx pair (2 PV matmuls)
                        psy = psY.tile([128, 512], f32, tag="psY", name="psy")
                        for jp in range(NT // 2):
                            for ql in range(2):
                                P = 2 * jp + ql
                                Np = (4 - P // 4) * 128
                                nc.tensor.matmul(
                                    psy[0:65, 0:Np],
                                    Vt[:, P, h, 0:65],
                                    expST[:, P, 0:Np],
                                    start=(P == 0), stop=(P == NT - 1),
                                    skip_group_check=True,
                                )
                            yield
                        yTb = ytp.tile([128, 512], bf16, tag="yT", name="yT")
                        if h % 2 == 0:
                            nc.vector.tensor_copy(yTb[0:65, :], psy[0:65, :])
                        else:
                            nc.scalar.copy(yTb[0:65, :], psy[0:65, :])
                        tpy = psTy.tile([128, 4, 66], bf16, tag="psTy", name="tpy")
                        for k in range(4):
                            nc.tensor.matmul(
                                tpy[:, k, 0:65], yTb[0:65, k * 128:(k + 1) * 128],
                                ident[0:65, 0:65], is_transpose=True,
                                start=True, stop=True,
                            )
                        if h % 2 == 0:
                            nc.scalar.copy(y_sb[:, :, h, :], tpy[:, :, 0:65])
                        else:
                            nc.vector.tensor_copy(y_sb[:, :, h, :], tpy[:, :, 0:65])
                        yield

                    # software pipeline: head h scores/exp woven with h-1's PV
                    exp_tiles = {}
                    prev_pv = None
                    for h in range(H):
                        exp_tiles[h] = expp.tile([128, NT, 512], bf16,
                                                 tag="expST", name="expST")
                        sc = emit_scores(h, exp_tiles[h])
                        for _ in sc:
                            if prev_pv is not None:
                                next(prev_pv, None)
                        if prev_pv is not None:
                            for _ in prev_pv:  # drain tail (yTb/y_sb copies)
                                pass
                        prev_pv = emit_pv(h, exp_tiles[h])
                    for _ in prev_pv:
                        pass

            # ===== Phase D/E/F: y-norm + residual, LN2, MLP =====
            with (
                tc.tile_pool(name="w1pool", bufs=1) as w1p,
                tc.tile_pool(name="x1nT", bufs=1) as x1nTp,
                tc.tile_pool(name="h1T8", bufs=1) as h1p,
                tc.tile_pool(name="ynorm", bufs=2) as ynp,
                tc.tile_pool(name="stat2", bufs=4) as stat2p,
                tc.tile_pool(name="x1nbf", bufs=2) as x1nbfp,
                tc.tile_pool(name="psT2", bufs=2, space="PSUM") as psT2,
            ):
                w1 = w1p.tile([128, NC, F], bf16, name="w1b")
                nc.sync.dma_start(w1[:], w1_d[:])
                x1nT = x1nTp.tile([128, NC, R], bf16)
                h1T8 = h1p.tile([128, JF, 2, R], fp8)

                for s in range(4):
                    k = 3 - s
                    den = ynp.tile([128, H], f32, tag="den", name="den")
                    nc.vector.tensor_copy(den[:], y_sb[:, k, :, 64])
                    rec = ynp.tile([128, H], f32, tag="rec", name="rec")
                    nc.vector.reciprocal(rec[:], den[:])
                    yf = ynp.tile([128, H, DH], f32, tag="yf", name="yf")
                    for hh in range(H):
                        if hh % 2 == 0:
                            nc.vector.tensor_scalar(
                                yf[:, hh, :], y_sb[:, k, hh, 0:64],
                                rec[:, hh:hh + 1], None, op0=ALU.mult,
                            )
                        else:
                            nc.scalar.activation(
                                yf[:, hh, :], y_sb[:, k, hh, 0:64],
                                AF.Identity, scale=rec[:, hh:hh + 1],
                            )
                    nc.vector.tensor_mul(x1[:, s, :], xn_keep[:, s, :], g1b[:])
                    nc.vector.tensor_add(x1[:, s, :], x1[:, s, :], b1rb[:])
                    nc.vector.tensor_add(
                        x1[:, s, :], x1[:, s, :],
                        yf[:].rearrange("p h d -> p (h d)"),
                    )
                    # LN2 for this slot
                    st6 = stat2p.tile([128, 2, 6], f32, tag="st6", name="st6b")
                    for g in range(2):
                        nc.vector.bn_stats(
                            st6[:, g, :], x1[:, s, g * 384:(g + 1) * 384]
                        )
                    st2 = stat2p.tile([128, 2], f32, tag="st2", name="st2b")
                    nc.vector.bn_aggr(st2[:], st6[:])
                    std = stat2p.tile([128, 1], f32, tag="std", name="stdb")
                    nc.scalar.activation(std[:], st2[:, 1:2], AF.Sqrt, bias=eps_t[:])
                    rstd = stat2p.tile([128, 1], f32, tag="rstd", name="rstdb")
                    nc.vector.reciprocal(rstd[:], std[:])
                    nmb = stat2p.tile([128, 1], f32, tag="nmb", name="nmbb")
                    nc.vector.tensor_scalar(
                        nmb[:], st2[:, 0:1], rstd[:], -1.0,
                        op0=ALU.mult, op1=ALU.mult,
                    )
                    x1n = x1nbfp.tile([128, C], bf16, tag="x1n", name="x1n")
                    nc.scalar.activation(
                        x1n[:], x1[:, s, :], AF.Identity, bias=nmb[:], scale=rstd[:]
                    )
                    tp = psT2.tile([128, NC, 128], bf16, tag="psT2", name="tpb")
                    for cb in range(NC):
                        nc.tensor.matmul(
                            tp[:, cb, :],
                            x1n[:, cb * 128:(cb + 1) * 128],
                            ident[:], is_transpose=True, start=True, stop=True,
                        )
                    if s % 2 == 0:
                        nc.vector.tensor_copy(
                            x1nT[:, :, s * 128:(s + 1) * 128], tp[:]
                        )
                    else:
                        nc.scalar.copy(x1nT[:, :, s * 128:(s + 1) * 128], tp[:])

                with (
                    tc.tile_pool(name="psH", bufs=2, space="PSUM") as psH,
                    tc.tile_pool(name="psO", bufs=2, space="PSUM") as psO,
                    tc.tile_pool(name="outp", bufs=2) as outp,
                ):
                    for nf in range(NF):
                        ps = psH.tile([128, 512], f32, tag="psH", name="psh")
                        for cb in range(NC):
                            nc.tensor.matmul(
                                ps[:], w1[:, cb, nf * 128:(nf + 1) * 128],
                                x1nT[:, cb, :],
                                start=(cb == 0), stop=(cb == NC - 1),
                            )
                        nc.scalar.activation(
                            h1T8[:, nf // 2, nf % 2, :], ps[:],
                            AF.Gelu, bias=b1s[:, nf:nf + 1],
                        )
                    for s in range(4):
                        o_sb = outp.tile([128, C], f32, tag="o", name="o_sb")
                        for g in range(2):
                            ps = psO.tile([128, 384], f32, tag="psO", name="pso")
                            for jf in range(JF):
                                nc.tensor.matmul(
                                    ps[:],
                                    h1T8[:, jf, :, s * 128:(s + 1) * 128],
                                    w2[:, jf, :, g * 384:(g + 1) * 384],
                                    start=(jf == 0), stop=False,
                                    perf_mode=PM.DoubleRow,
                                    skip_group_check=True,
                                )
                            nc.tensor.matmul(
                                ps[:], ones1[:], b2row[:, g * 384:(g + 1) * 384],
                                start=False, stop=True, skip_group_check=True,
                            )
                            nc.vector.scalar_tensor_tensor(
                                o_sb[:, g * 384:(g + 1) * 384], ps[:], 1.0 / SW,
                                x1[:, s, g * 384:(g + 1) * 384],
                                op0=ALU.mult, op1=ALU.add,
                            )
                        nc.sync.dma_start(out_d[s * 128:(s + 1) * 128, :], o_sb[:])

    nc.compile()
    return nc


def _prep_shared(inputs):
    import ml_dtypes

    f = np.float32
    bf = ml_dtypes.bfloat16
    f8 = ml_dtypes.float8_e4m3
    g1 = np.asarray(inputs["ln1_g"], f)
    b1r = np.asarray(inputs["ln1_b"], f)
    g2 = np.asarray(inputs["ln2_g"], f)
    b2r = np.asarray(inputs["ln2_b"], f)
    Wq, Wk, Wv = (np.asarray(inputs[k], f) for k in ("Wq", "Wk", "Wv"))
    W1, W2 = np.asarray(inputs["W1"], f), np.asarray(inputs["W2"], f)

    def dr_pack(w, scale):
        # [K, M] -> [128, K/256, 2, M] with channel k = j*256 + q*128 + p
        K, M = w.shape
        return np.ascontiguousarray(
            (w * scale).reshape(K // 256, 2, 128, M).transpose(2, 0, 1, 3)
        ).astype(f8)

    def bf_pack(w):
        # [K, M] -> [128, K/128, M]
        K, M = w.shape
        return np.ascontiguousarray(
            w.reshape(K // 128, 128, M).transpose(1, 0, 2)
        ).astype(bf)

    def colmajor_bias(b, n):
        return np.ascontiguousarray(b.reshape(n, 128).T)

    bv_eff = b1r @ Wv + np.asarray(inputs["bv"], f)
    rows = np.arange(128)
    trimask = np.where(rows[:, None] > rows[None, :], -1e5, 0.0).astype(bf)

    return {
        "wq8": dr_pack(g1[:, None] * Wq, SW),
        "wk8": dr_pack(g1[:, None] * Wk, SW),
        "wv8": dr_pack(g1[:, None] * Wv, SW),
        "w1b": bf_pack(g2[:, None] * W1),
        "w28": dr_pack(W2, SW),
        "bq": colmajor_bias(b1r @ Wq + np.asarray(inputs["bq"], f), HP),
        "bk": colmajor_bias(b1r @ Wk + np.asarray(inputs["bk"], f), HP),
        "b1": colmajor_bias(b2r @ W1 + np.asarray(inputs["b1"], f), NF),
        "b2row": np.ascontiguousarray(np.asarray(inputs["b2"], f)[None, :]).astype(bf),
        "g1b": np.ascontiguousarray(np.broadcast_to(g1, (128, C))).astype(bf),
        "b1rb": np.ascontiguousarray(np.broadcast_to(b1r + bv_eff, (128, C))).astype(f),
        "trimask": np.ascontiguousarray(trimask),
        "identb": np.eye(128, dtype=f).astype(bf),
    }


def kernel(**inputs):
    from concourse.bass_utils import run_bass_kernel_spmd

    if "nc" not in _CACHE:
        _CACHE["nc"] = _build_program()
    nc = _CACHE["nc"]

    x = np.asarray(inputs["x"], np.float32)
    shared = _prep_shared(inputs)

    in_maps = []
    for c8 in range(8):
        b, c = c8 // 4, c8 % 4
        pad = 3 - c
        x_ctx = np.zeros((T, C), np.float32)
        x_ctx[pad * 128:] = x[b, 0:(13 + c) * 128]
        valid = np.zeros(NT, np.float32)
        valid[pad:] = 1.0
        m = dict(shared)
        m["x_ctx"] = x_ctx
        m["validv"] = np.ascontiguousarray(
            np.broadcast_to(valid * (1.0 / SXW), (128, NT)).astype(np.float32))
        m["vones"] = np.ascontiguousarray(
            np.broadcast_to(valid, (128, NT)).astype(np.float32))
        in_maps.append(m)

    trace = bool(int(os.environ.get("KERNEL_TRACE", "0")))
    try:
        res = run_bass_kernel_spmd(nc, in_maps, core_ids=list(range(8)), trace=trace)
    except ModuleNotFoundError:
        res = run_bass_kernel_spmd(nc, in_maps, core_ids=list(range(8)), trace=False)
    _CACHE["last_result"] = res

    out = np.empty((B, T, C), np.float32)
    for c8 in range(8):
        b, c = c8 // 4, c8 % 4
        for s in range(4):
            blk = c + 4 * s
            out[b, blk * 128:(blk + 1) * 128] = \
                res.results[c8]["out"][s * 128:(s + 1) * 128]
    return out


# revision 11
# speedup vs baseline: 1.1087x; 1.1087x over previous
"""Trainium2 Bass kernel for a GPT-style decoder block (B=2, T=2048, C=768, H=12).

Sharding: 8 cores = 2 batches x 4 interleaved block-sets. Core c owns 128-row
blocks {c, c+4, c+8, c+12} of its batch. Its context buffer holds the 16
position-blocks [zeros x (3-c) | blocks 0..12+c]; the own blocks then sit at
the STATIC positions {3, 7, 11, 15} with causal context = position prefixes of
length {4, 8, 12, 16} blocks. Every core therefore runs the same instruction
stream while doing the load-balanced share (40/64) of the causal attention
work; the inserted zero blocks are masked via a per-block validity scale on V
(and on the denominator ones-column), so they contribute exactly 0 to both the
attention numerator and the softmax denominator.

Numerics: Q/K/V projections and the second MLP matmul run in fp8e4 with
DoubleRow perf mode (two 128-channel contraction chunks per instruction);
scores, P, V and the first MLP matmul stay bf16 (fp8 everywhere pushes the
fixed-seed rel-err past the 2e-2 gate; this mix measures ~1.6e-2 in numpy).
LN statistics, softmax normalization, residuals and the output stay fp32.
fp8 scales (weights x512, activations x16) are divided out on PSUM->SBUF
copies.

Note: reference computes scores = K @ Q^T, so the output-row operand is K (own
rows) and the context operand is Q/V. The causal triangle on each own block's
diagonal position is applied by accumulating a -1e5 upper-triangle mask into
the scores PSUM with one extra bf16 matmul. V / Q projections are interleaved
into the LN1 loop so TensorE stays busy during the per-tile LN chains.
"""

import os

import numpy as np

B, T, C = 2, 2048, 768
H, DH = 12, 64
F = 4 * C
R = 512            # own rows per core
NT = 16            # ctx position blocks
NC = C // 128      # 6
JC = NC // 2       # 3 channel pairs
NF = F // 128      # 24
JF = NF // 2       # 12 hidden pairs
HP = H // 2        # 6 head pairs
VS = 66            # per-head stride in Vt (64 v + 1 ones + pad)
EPS = 1e-3
SX = 16.0          # fp8 scale on normalized activations
SW = 512.0         # fp8 scale on weights
SXW = SX * SW      # 8192

_CACHE = {}


def _build_program():
    import concourse.bass as bass  # noqa: F401
    import concourse.mybir as mybir
    import concourse.tile as tile
    from concourse import bacc

    dt = mybir.dt
    f32 = dt.float32
    bf16 = dt.bfloat16
    fp8 = dt.float8e4
    AF = mybir.ActivationFunctionType
    ALU = mybir.AluOpType
    PM = mybir.MatmulPerfMode

    nc = bacc.Bacc("TRN2", target_bir_lowering=False, debug=False, num_devices=8)

    # ---- DRAM I/O ----
    x_ctx = nc.dram_tensor("x_ctx", [T, C], f32, kind="ExternalInput")
    validv_d = nc.dram_tensor("validv", [128, NT], f32, kind="ExternalInput")
    vones_d = nc.dram_tensor("vones", [128, NT], f32, kind="ExternalInput")
    wq_d = nc.dram_tensor("wq8", [128, JC, 2, C], fp8, kind="ExternalInput")
    wk_d = nc.dram_tensor("wk8", [128, JC, 2, C], fp8, kind="ExternalInput")
    wv_d = nc.dram_tensor("wv8", [128, JC, 2, C], fp8, kind="ExternalInput")
    w1_d = nc.dram_tensor("w1b", [128, NC, F], bf16, kind="ExternalInput")
    w2_d = nc.dram_tensor("w28", [128, JF, 2, C], fp8, kind="ExternalInput")
    bq_d = nc.dram_tensor("bq", [128, HP], f32, kind="ExternalInput")
    bk_d = nc.dram_tensor("bk", [128, HP], f32, kind="ExternalInput")
    b1_d = nc.dram_tensor("b1", [128, NF], f32, kind="ExternalInput")
    b2_d = nc.dram_tensor("b2row", [1, C], bf16, kind="ExternalInput")
    g1b_d = nc.dram_tensor("g1b", [128, C], bf16, kind="ExternalInput")
    b1rb_d = nc.dram_tensor("b1rb", [128, C], f32, kind="ExternalInput")
    tri_d = nc.dram_tensor("trimask", [128, 128], bf16, kind="ExternalInput")
    ident_d = nc.dram_tensor("identb", [128, 128], bf16, kind="ExternalInput")
    out_d = nc.dram_tensor("out", [R, C], f32, kind="ExternalOutput")

    OWN = (3, 7, 11, 15)  # own position blocks (slot s -> position 4s+3)

    with tile.TileContext(nc) as tc:
        with (
            tc.tile_pool(name="const", bufs=1) as constp,
            tc.tile_pool(name="keep", bufs=1) as keepp,
            tc.tile_pool(name="w2pool", bufs=1) as w2p,
        ):
            validv = constp.tile([128, NT], f32)
            nc.sync.dma_start(validv[:], validv_d[:])
            vones = constp.tile([128, NT], f32)
            nc.sync.dma_start(vones[:], vones_d[:])
            tri = constp.tile([128, 128], bf16)
            nc.sync.dma_start(tri[:], tri_d[:])
            ident = constp.tile([128, 128], bf16)
            nc.sync.dma_start(ident[:], ident_d[:])
            bqs = constp.tile([128, HP], f32)
            nc.sync.dma_start(bqs[:], bq_d[:])
            bks = constp.tile([128, HP], f32)
            nc.sync.dma_start(bks[:], bk_d[:])
            b1s = constp.tile([128, NF], f32)
            nc.sync.dma_start(b1s[:], b1_d[:])
            b2row = constp.tile([1, C], bf16)
            nc.sync.dma_start(b2row[:], b2_d[:])
            g1b = constp.tile([128, C], bf16)
            nc.sync.dma_start(g1b[:], g1b_d[:])
            b1rb = constp.tile([128, C], f32)
            nc.sync.dma_start(b1rb[:], b1rb_d[:])
            ones1 = constp.tile([1, 128], bf16)
            nc.vector.memset(ones1[:], 1.0)
            onesf = constp.tile([128, H, 1], f32)
            nc.vector.memset(onesf[:], 1.0)
            eps_t = constp.tile([128, 1], f32)
            nc.vector.memset(eps_t[:], EPS)

            # w2 (fp8, small) arrives early so its DMA overlaps everything
            w2 = w2p.tile([128, JF, 2, C], fp8, name="w28")
            nc.sync.dma_start(w2[:], w2_d[:])

            xn_keep = keepp.tile([128, 4, C], f32)   # own rows (slot order), fp32
            x1 = keepp.tile([128, 4, C], f32)        # post-attention residual
            y_sb = keepp.tile([128, 4, H, 65], bf16)  # y token-major; k=3-s order

            with (
                tc.tile_pool(name="xnT8", bufs=1) as xnT8p,
                tc.tile_pool(name="QT", bufs=1) as QTp,
                tc.tile_pool(name="KT", bufs=1) as KTp,
                tc.tile_pool(name="V", bufs=1) as Vp,
            ):
                xnT8 = xnT8p.tile([128, JC, 2, T], fp8)       # xn^T * 16
                QT = QTp.tile([128, HP, T], bf16)             # q (true scale)
                KT = KTp.tile([128, HP, R], bf16)             # k own, col k=3-s
                xnT8own = KTp.tile([128, JC, 2, R], fp8)
                Vt = Vp.tile([128, NT, H, VS], bf16)          # v true, [..,64]=1

                # ===== Phase A+B: LN1, transpose, QKV (interleaved) =====
                with (
                    tc.tile_pool(name="xin", bufs=3) as xinp,
                    tc.tile_pool(name="stat", bufs=4) as statp,
                    tc.tile_pool(name="xnbf", bufs=3) as xnbfp,
                    tc.tile_pool(name="wqkv", bufs=1) as wp,
                    tc.tile_pool(name="psT", bufs=2, space="PSUM") as psT,
                    tc.tile_pool(name="psQ", bufs=2, space="PSUM") as psQ,
                    tc.tile_pool(name="psV", bufs=2, space="PSUM") as psV,
                ):
                    wq = wp.tile([128, JC, 2, C], fp8, name="wq8")
                    nc.sync.dma_start(wq[:], wq_d[:])
                    wk = wp.tile([128, JC, 2, C], fp8, name="wk8")
                    nc.sync.dma_start(wk[:], wk_d[:])
                    wv = wp.tile([128, JC, 2, C], fp8, name="wv8")
                    nc.sync.dma_start(wv[:], wv_d[:])

                    TILE_ORDER = (3, 7, 11, 15, 0, 1, 2, 4, 5, 6,
                                  8, 9, 10, 12, 13, 14)
                    qdone = [False] * 4
                    done = set()

                    def emit_front(tb):
                        # DMA + LN stats + normalize (vector/ACT, no PSUM)
                        # (emitted inline, in tile order)
                        xt = xinp.tile([128, C], f32, tag="xt", name="xt")
                        nc.sync.dma_start(xt[:], x_ctx[tb * 128:(tb + 1) * 128, :])
                        st6 = statp.tile([128, 2, 6], f32, tag="st6", name="st6")
                        for g in range(2):
                            nc.vector.bn_stats(
                                st6[:, g, :], xt[:, g * 384:(g + 1) * 384]
                            )
                        st2 = statp.tile([128, 2], f32, tag="st2", name="st2")
                        nc.vector.bn_aggr(st2[:], st6[:])
                        std = statp.tile([128, 1], f32, tag="std", name="std")
                        nc.scalar.activation(std[:], st2[:, 1:2], AF.Sqrt, bias=eps_t[:])
                        rstd = statp.tile([128, 1], f32, tag="rstd", name="rstd")
                        nc.vector.reciprocal(rstd[:], std[:])
                        nmb = statp.tile([128, 1], f32, tag="nmb", name="nmb")
                        nc.vector.tensor_scalar(
                            nmb[:], st2[:, 0:1], rstd[:], -1.0,
                            op0=ALU.mult, op1=ALU.mult,
                        )
                        xn_bf = xnbfp.tile([128, C], bf16, tag="xn_bf", name="xn_bf")
                        nc.scalar.activation(
                            xn_bf[:], xt[:], AF.Identity, bias=nmb[:], scale=rstd[:]
                        )
                        if tb in OWN:
                            s = OWN.index(tb)
                            nc.vector.tensor_scalar(
                                xn_keep[:, s, :], xt[:], st2[:, 0:1], rstd[:],
                                op0=ALU.subtract, op1=ALU.mult,
                            )
                        return xn_bf

                    def emit_back(tb, xn_bf):
                        done.add(tb)
                        tp = psT.tile([128, JC, 2, 128], bf16, tag="psT", name="tp")
                        for cb in range(NC):
                            nc.tensor.matmul(
                                tp[:, cb // 2, cb % 2, :],
                                xn_bf[:, cb * 128:(cb + 1) * 128],
                                ident[:], is_transpose=True, start=True, stop=True,
                            )
                        if tb % 2 == 0:
                            nc.vector.tensor_scalar(
                                xnT8[:, :, :, tb * 128:(tb + 1) * 128], tp[:],
                                SX, None, op0=ALU.mult,
                            )
                        else:
                            nc.scalar.mul(
                                xnT8[:, :, :, tb * 128:(tb + 1) * 128], tp[:], SX
                            )

                        # V projection for this tile (fp8 DoubleRow)
                        for g in range(2):
                            ps = psV.tile([128, 6, 64], f32, tag="psV", name="psv")
                            for j in range(JC):
                                nc.tensor.matmul(
                                    ps[:], xnT8[:, j, :, tb * 128:(tb + 1) * 128],
                                    wv[:, j, :, g * 384:(g + 1) * 384],
                                    start=(j == 0), stop=(j == JC - 1),
                                    perf_mode=PM.DoubleRow,
                                )
                            if (tb + g) % 2 == 0:
                                nc.vector.tensor_scalar(
                                    Vt[:, tb, g * 6:(g + 1) * 6, 0:64],
                                    ps[:], validv[:, tb:tb + 1], None, op0=ALU.mult,
                                )
                            else:
                                nc.scalar.activation(
                                    Vt[:, tb, g * 6:(g + 1) * 6, 0:64], ps[:],
                                    AF.Identity, scale=validv[:, tb:tb + 1],
                                )
                        nc.vector.tensor_scalar(
                            Vt[:, tb, :, 64:65], onesf[:],
                            vones[:, tb:tb + 1], None, op0=ALU.mult,
                        )
                        if tb in OWN:
                            s = OWN.index(tb)
                            nc.gpsimd.tensor_copy(
                                xnT8own[:, :, :, (3 - s) * 128:(4 - s) * 128],
                                xnT8[:, :, :, tb * 128:(tb + 1) * 128],
                            )
                        if all(p in done for p in OWN) and not qdone[0] \
                                and tb == 15:
                            qdone[0] = True
                            for hp in range(HP):
                                ps = psQ.tile([128, 512], f32, tag="psQ", name="psk")
                                for j in range(JC):
                                    nc.tensor.matmul(
                                        ps[:], wk[:, j, :, hp * 128:(hp + 1) * 128],
                                        xnT8own[:, j, :, :],
                                        start=(j == 0), stop=(j == JC - 1),
                                        perf_mode=PM.DoubleRow,
                                    )
                                nc.vector.tensor_scalar(
                                    KT[:, hp, :], ps[:], 1.0 / SXW, bks[:, hp:hp + 1],
                                    op0=ALU.mult, op1=ALU.add,
                                )
                        for nb in range(4):
                            grp = {4 * nb, 4 * nb + 1, 4 * nb + 2, 4 * nb + 3}
                            key = "q%d" % nb
                            if not (grp <= done) or key in done:
                                continue
                            done.add(key)
                            for hp in range(HP):
                                ps = psQ.tile([128, 512], f32, tag="psQ", name="psq")
                                for j in range(JC):
                                    nc.tensor.matmul(
                                        ps[:], wq[:, j, :, hp * 128:(hp + 1) * 128],
                                        xnT8[:, j, :, nb * 512:(nb + 1) * 512],
                                        start=(j == 0), stop=(j == JC - 1),
                                        perf_mode=PM.DoubleRow,
                                    )
                                if hp % 2 == 0:
                                    nc.scalar.activation(
                                        QT[:, hp, nb * 512:(nb + 1) * 512], ps[:],
                                        AF.Identity, bias=bqs[:, hp:hp + 1],
                                        scale=1.0 / SXW,
                                    )
                                else:
                                    nc.vector.tensor_scalar(
                                        QT[:, hp, nb * 512:(nb + 1) * 512], ps[:],
                                        1.0 / SXW, bqs[:, hp:hp + 1],
                                        op0=ALU.mult, op1=ALU.add,
                                    )

                    for tb in TILE_ORDER:
                        emit_back(tb, emit_front(tb))

                # ===== Phase C: attention =====
                with (
                    tc.tile_pool(name="exps", bufs=2) as expp,
                    tc.tile_pool(name="yT", bufs=2) as ytp,
                    tc.tile_pool(name="psS", bufs=2, space="PSUM") as psS,
                    tc.tile_pool(name="psY", bufs=2, space="PSUM") as psY,
                    tc.tile_pool(name="psTy", bufs=2, space="PSUM") as psTy,
                ):
                    def emit_scores(h, expST):
                        # generator: one step per ctx pair (scores + exp)
                        hp, off = h // 2, 64 * (h % 2)
                        for jp in range(NT // 2):
                            Np = (4 - jp // 2) * 128
                            ps = psS.tile([128, 2, 512], f32, tag="psS", name="pss")
                            for ql in range(2):
                                P = 2 * jp + ql
                                diag = (P % 4 == 3)
                                nc.tensor.matmul(
                                    ps[:, ql, 0:Np],
                                    QT[off:off + 64, hp, P * 128:(P + 1) * 128],
                                    KT[off:off + 64, hp, 0:Np],
                                    start=True, stop=not diag,
                                    skip_group_check=diag,
                                )
                                if diag:
                                    nc.tensor.matmul(
                                        ps[:, ql, Np - 128:Np],
                                        ident[:], tri[:],
                                        start=False, stop=True,
                                        skip_group_check=True,
                                    )
                            nc.scalar.activation(
                                expST[:, 2 * jp:2 * jp + 2, 0:Np], ps[:, :, 0:Np],
                                AF.Exp, scale=0.125,
                            )
                            yield

                    def emit_pv(h, expST):
                        # generator: one step per ctx pair (2 PV matmuls)
                        psy = psY.tile([128, 512], f32, tag="psY", name="psy")
                        for jp in range(NT // 2):
                            for ql in range(2):
                                P = 2 * jp + ql
                                Np = (4 - P // 4) * 128
                                nc.tensor.matmul(
                                    psy[0:65, 0:Np],
                                    Vt[:, P, h, 0:65],
                                    expST[:, P, 0:Np],
                                    start=(P == 0), stop=(P == NT - 1),
                                    skip_group_check=True,
                                )
                            yield
                        yTb = ytp.tile([128, 512], bf16, tag="yT", name="yT")
                        if h % 2 == 0:
                            nc.vector.tensor_copy(yTb[0:65, :], psy[0:65, :])
                        else:
                            nc.scalar.copy(yTb[0:65, :], psy[0:65, :])
                        tpy = psTy.tile([128, 4, 66], bf16, tag="psTy", name="tpy")
                        for k in range(4):
                            nc.tensor.matmul(
                                tpy[:, k, 0:65], yTb[0:65, k * 128:(k + 1) * 128],
                                ident[0:65, 0:65], is_transpose=True,
                                start=True, stop=True,
                            )
                        if h % 2 == 0:
                            nc.scalar.copy(y_sb[:, :, h, :], tpy[:, :, 0:65])
                        else:
                            nc.vector.tensor_copy(y_sb[:, :, h, :], tpy[:, :, 0:65])
                        yield

                    # software pipeline: head h scores/exp woven with h-1's PV
                    exp_tiles = {}
                    prev_pv = None
                    for h in range(H):
                        exp_tiles[h] = expp.tile([128, NT, 512], bf16,
                                                 tag="expST", name="expST")
                        sc = emit_scores(h, exp_tiles[h])
                        for _ in sc:
                            if prev_pv is not None:
                                next(prev_pv, None)
                        if prev_pv is not None:
                            for _ in prev_pv:  # drain tail (yTb/y_sb copies)
                                pass
                        prev_pv = emit_pv(h, exp_tiles[h])
                    for _ in prev_pv:
                        pass

            # ===== Phase D/E/F: y-norm + residual, LN2, MLP =====
            with (
                tc.tile_pool(name="w1pool", bufs=1) as w1p,
                tc.tile_pool(name="x1nT", bufs=1) as x1nTp,
                tc.tile_pool(name="h1T8", bufs=1) as h1p,
                tc.tile_pool(name="ynorm", bufs=2) as ynp,
                tc.tile_pool(name="stat2", bufs=4) as stat2p,
                tc.tile_pool(name="x1nbf", bufs=2) as x1nbfp,
                tc.tile_pool(name="psT2", bufs=2, space="PSUM") as psT2,
            ):
                w1 = w1p.tile([128, NC, F], bf16, name="w1b")
                nc.sync.dma_start(w1[:], w1_d[:])
                x1nT = x1nTp.tile([128, NC, R], bf16)
                h1T8 = h1p.tile([128, JF, 2, R], fp8)

                for s in range(4):
                    k = 3 - s
                    den = ynp.tile([128, H], f32, tag="den", name="den")
                    nc.vector.tensor_copy(den[:], y_sb[:, k, :, 64])
                    rec = ynp.tile([128, H], f32, tag="rec", name="rec")
                    nc.vector.reciprocal(rec[:], den[:])
                    yf = ynp.tile([128, H, DH], f32, tag="yf", name="yf")
                    for hh in range(H):
                        if hh % 2 == 0:
                            nc.vector.tensor_scalar(
                                yf[:, hh, :], y_sb[:, k, hh, 0:64],
                                rec[:, hh:hh + 1], None, op0=ALU.mult,
                            )
                        else:
                            nc.scalar.activation(
                                yf[:, hh, :], y_sb[:, k, hh, 0:64],
                                AF.Identity, scale=rec[:, hh:hh + 1],
                            )
                    nc.vector.tensor_mul(x1[:, s, :], xn_keep[:, s, :], g1b[:])
                    nc.vector.tensor_add(x1[:, s, :], x1[:, s, :], b1rb[:])
                    nc.vector.tensor_add(
                        x1[:, s, :], x1[:, s, :],
                        yf[:].rearrange("p h d -> p (h d)"),
                    )
                    # LN2 for this slot
                    st6 = stat2p.tile([128, 2, 6], f32, tag="st6", name="st6b")
                    for g in range(2):
                        nc.vector.bn_stats(
                            st6[:, g, :], x1[:, s, g * 384:(g + 1) * 384]
                        )
                    st2 = stat2p.tile([128, 2], f32, tag="st2", name="st2b")
                    nc.vector.bn_aggr(st2[:], st6[:])
                    std = stat2p.tile([128, 1], f32, tag="std", name="stdb")
                    nc.scalar.activation(std[:], st2[:, 1:2], AF.Sqrt, bias=eps_t[:])
                    rstd = stat2p.tile([128, 1], f32, tag="rstd", name="rstdb")
                    nc.vector.reciprocal(rstd[:], std[:])
                    nmb = stat2p.tile([128, 1], f32, tag="nmb", name="nmbb")
                    nc.vector.tensor_scalar(
                        nmb[:], st2[:, 0:1], rstd[:], -1.0,
                        op0=ALU.mult, op1=ALU.mult,
                    )
                    x1n = x1nbfp.tile([128, C], bf16, tag="x1n", name="x1n")
                    nc.scalar.activation(
                        x1n[:], x1[:, s, :], AF.Identity, bias=nmb[:], scale=rstd[:]
                    )
                    tp = psT2.tile([128, NC, 128], bf16, tag="psT2", name="tpb")
                    for cb in range(NC):
                        nc.tensor.matmul(
                            tp[:, cb, :],
                            x1n[:, cb * 128:(cb + 1) * 128],
                            ident[:], is_transpose=True, start=True, stop=True,
                        )
                    if s % 2 == 0:
                        nc.vector.tensor_copy(
                            x1nT[:, :, s * 128:(s + 1) * 128], tp[:]
                        )
                    else:
                        nc.scalar.copy(x1nT[:, :, s * 128:(s + 1) * 128], tp[:])

                with (
                    tc.tile_pool(name="psH", bufs=2, space="PSUM") as psH,
                    tc.tile_pool(name="psO", bufs=2, space="PSUM") as psO,
                    tc.tile_pool(name="outp", bufs=2) as outp,
                ):
                    for nf in range(NF):
                        ps = psH.tile([128, 512], f32, tag="psH", name="psh")
                        for cb in range(NC):
                            nc.tensor.matmul(
                                ps[:], w1[:, cb, nf * 128:(nf + 1) * 128],
                                x1nT[:, cb, :],
                                start=(cb == 0), stop=(cb == NC - 1),
                            )
                        nc.scalar.activation(
                            h1T8[:, nf // 2, nf % 2, :], ps[:],
                            AF.Gelu, bias=b1s[:, nf:nf + 1],
                        )
                    for s in range(4):
                        o_sb = outp.tile([128, C], f32, tag="o", name="o_sb")
                        for g in range(2):
                            ps = psO.tile([128, 384], f32, tag="psO", name="pso")
                            for jf in range(JF):
                                nc.tensor.matmul(
                                    ps[:],
                                    h1T8[:, jf, :, s * 128:(s + 1) * 128],
                                    w2[:, jf, :, g * 384:(g + 1) * 384],
                                    start=(jf == 0), stop=False,
                                    perf_mode=PM.DoubleRow,
                                    skip_group_check=True,
                                )
                            nc.tensor.matmul(
                                ps[:], ones1[:], b2row[:, g * 384:(g + 1) * 384],
                                start=False, stop=True, skip_group_check=True,
                            )
                            nc.vector.scalar_tensor_tensor(
                                o_sb[:, g * 384:(g + 1) * 384], ps[:], 1.0 / SW,
                                x1[:, s, g * 384:(g + 1) * 384],
                                op0=ALU.mult, op1=ALU.add,
                            )
                        nc.sync.dma_start(out_d[s * 128:(s + 1) * 128, :], o_sb[:])

    nc.compile()
    return nc


def _prep_shared(inputs):
    import ml_dtypes

    f = np.float32
    bf = ml_dtypes.bfloat16
    f8 = ml_dtypes.float8_e4m3
    g1 = np.asarray(inputs["ln1_g"], f)
    b1r = np.asarray(inputs["ln1_b"], f)
    g2 = np.asarray(inputs["ln2_g"], f)
    b2r = np.asarray(inputs["ln2_b"], f)
    Wq, Wk, Wv = (np.asarray(inputs[k], f) for k in ("Wq", "Wk", "Wv"))
    W1, W2 = np.asarray(inputs["W1"], f), np.asarray(inputs["W2"], f)

    def dr_pack(w, scale):
        # [K, M] -> [128, K/256, 2, M] with channel k = j*256 + q*128 + p
        K, M = w.shape
        return np.ascontiguousarray(
            (w * scale).reshape(K // 256, 2, 128, M).transpose(2, 0, 1, 3)
        ).astype(f8)

    def bf_pack(w):
        # [K, M] -> [128, K/128, M]
        K, M = w.shape
        return np.ascontiguousarray(
            w.reshape(K // 128, 128, M).transpose(1, 0, 2)
        ).astype(bf)

    def colmajor_bias(b, n):
        return np.ascontiguousarray(b.reshape(n, 128).T)

    bv_eff = b1r @ Wv + np.asarray(inputs["bv"], f)
    rows = np.arange(128)
    trimask = np.where(rows[:, None] > rows[None, :], -1e5, 0.0).astype(bf)

    return {
        "wq8": dr_pack(g1[:, None] * Wq, SW),
        "wk8": dr_pack(g1[:, None] * Wk, SW),
        "wv8": dr_pack(g1[:, None] * Wv, SW),
        "w1b": bf_pack(g2[:, None] * W1),
        "w28": dr_pack(W2, SW),
        "bq": colmajor_bias(b1r @ Wq + np.asarray(inputs["bq"], f), HP),
        "bk": colmajor_bias(b1r @ Wk + np.asarray(inputs["bk"], f), HP),
        "b1": colmajor_bias(b2r @ W1 + np.asarray(inputs["b1"], f), NF),
        "b2row": np.ascontiguousarray(np.asarray(inputs["b2"], f)[None, :]).astype(bf),
        "g1b": np.ascontiguousarray(np.broadcast_to(g1, (128, C))).astype(bf),
        "b1rb": np.ascontiguousarray(np.broadcast_to(b1r + bv_eff, (128, C))).astype(f),
        "trimask": np.ascontiguousarray(trimask),
        "identb": np.eye(128, dtype=f).astype(bf),
    }


def kernel(**inputs):
    from concourse.bass_utils import run_bass_kernel_spmd

    if "nc" not in _CACHE:
        _CACHE["nc"] = _build_program()
    nc = _CACHE["nc"]

    x = np.asarray(inputs["x"], np.float32)
    shared = _prep_shared(inputs)

    in_maps = []
    for c8 in range(8):
        b, c = c8 // 4, c8 % 4
        pad = 3 - c
        x_ctx = np.zeros((T, C), np.float32)
        x_ctx[pad * 128:] = x[b, 0:(13 + c) * 128]
        valid = np.zeros(NT, np.float32)
        valid[pad:] = 1.0
        m = dict(shared)
        m["x_ctx"] = x_ctx
        m["validv"] = np.ascontiguousarray(
            np.broadcast_to(valid * (1.0 / SXW), (128, NT)).astype(np.float32))
        m["vones"] = np.ascontiguousarray(
            np.broadcast_to(valid, (128, NT)).astype(np.float32))
        in_maps.append(m)

    trace = bool(int(os.environ.get("KERNEL_TRACE", "0")))
    try:
        res = run_bass_kernel_spmd(nc, in_maps, core_ids=list(range(8)), trace=trace)
    except ModuleNotFoundError:
        res = run_bass_kernel_spmd(nc, in_maps, core_ids=list(range(8)), trace=False)
    _CACHE["last_result"] = res

    out = np.empty((B, T, C), np.float32)
    for c8 in range(8):
        b, c = c8 // 4, c8 % 4
        for s in range(4):
            blk = c + 4 * s
            out[b, blk * 128:(blk + 1) * 128] = \
                res.results[c8]["out"][s * 128:(s + 1) * 128]
    return out


# revision 13
# speedup vs baseline: 1.1964x; 1.0791x over previous
"""Trainium2 Bass kernel for a GPT-style decoder block (B=2, T=2048, C=768, H=12).

Sharding: 8 cores = 2 batches x 4 interleaved block-sets. Core c owns 128-row
blocks {c, c+4, c+8, c+12} of its batch. Its context buffer holds the 16
position-blocks [zeros x (3-c) | blocks 0..12+c]; the own blocks then sit at
the STATIC positions {3, 7, 11, 15} with causal context = position prefixes of
length {4, 8, 12, 16} blocks. Every core therefore runs the same instruction
stream while doing the load-balanced share (40/64) of the causal attention
work; the inserted zero blocks are masked via a per-block validity scale on V
(and on the denominator ones-column), so they contribute exactly 0 to both the
attention numerator and the softmax denominator.

Numerics: Q/K/V projections and the second MLP matmul run in fp8e4 with
DoubleRow perf mode (two 128-channel contraction chunks per instruction);
scores, P, V and the first MLP matmul stay bf16 (fp8 everywhere pushes the
fixed-seed rel-err past the 2e-2 gate; this mix measures ~1.6e-2 in numpy).
LN statistics, softmax normalization, residuals and the output stay fp32.
fp8 scales (weights x512, activations x16) are divided out on PSUM->SBUF
copies.

Note: reference computes scores = K @ Q^T, so the output-row operand is K (own
rows) and the context operand is Q/V. The causal triangle on each own block's
diagonal position is applied by accumulating a -1e5 upper-triangle mask into
the scores PSUM with one extra bf16 matmul. V / Q projections are interleaved
into the LN1 loop so TensorE stays busy during the per-tile LN chains.
"""

import os

import numpy as np

B, T, C = 2, 2048, 768
H, DH = 12, 64
F = 4 * C
R = 512            # own rows per core
NT = 16            # ctx position blocks
NC = C // 128      # 6
JC = NC // 2       # 3 channel pairs
NF = F // 128      # 24
JF = NF // 2       # 12 hidden pairs
HP = H // 2        # 6 head pairs
VS = 66            # per-head stride in Vt (64 v + 1 ones + pad)
EPS = 1e-3
SX = 16.0          # fp8 scale on normalized activations
SW = 512.0         # fp8 scale on weights
SXW = SX * SW      # 8192

_CACHE = {}


def _build_program():
    import concourse.bass as bass  # noqa: F401
    import concourse.mybir as mybir
    import concourse.tile as tile
    from concourse import bacc

    dt = mybir.dt
    f32 = dt.float32
    bf16 = dt.bfloat16
    fp8 = dt.float8e4
    AF = mybir.ActivationFunctionType
    ALU = mybir.AluOpType
    PM = mybir.MatmulPerfMode

    nc = bacc.Bacc("TRN2", target_bir_lowering=False, debug=False, num_devices=8)

    # ---- DRAM I/O ----
    x_ctx = nc.dram_tensor("x_ctx", [T, C], f32, kind="ExternalInput")
    validv_d = nc.dram_tensor("validv", [128, NT], f32, kind="ExternalInput")
    vones_d = nc.dram_tensor("vones", [128, NT], f32, kind="ExternalInput")
    wq_d = nc.dram_tensor("wq8", [128, JC, 2, C], fp8, kind="ExternalInput")
    wk_d = nc.dram_tensor("wk8", [128, JC, 2, C], fp8, kind="ExternalInput")
    wv_d = nc.dram_tensor("wv8", [128, JC, 2, C], fp8, kind="ExternalInput")
    w1_d = nc.dram_tensor("w1b", [128, NC, F], bf16, kind="ExternalInput")
    w2_d = nc.dram_tensor("w28", [128, JF, 2, C], fp8, kind="ExternalInput")
    bq_d = nc.dram_tensor("bq", [128, HP], f32, kind="ExternalInput")
    bk_d = nc.dram_tensor("bk", [128, HP], f32, kind="ExternalInput")
    b1_d = nc.dram_tensor("b1", [128, NF], f32, kind="ExternalInput")
    b2_d = nc.dram_tensor("b2row", [1, C], bf16, kind="ExternalInput")
    g1b_d = nc.dram_tensor("g1b", [128, C], bf16, kind="ExternalInput")
    b1rb_d = nc.dram_tensor("b1rb", [128, C], f32, kind="ExternalInput")
    tri_d = nc.dram_tensor("trimask", [128, 128], bf16, kind="ExternalInput")
    ident_d = nc.dram_tensor("identb", [128, 128], bf16, kind="ExternalInput")
    out_d = nc.dram_tensor("out", [R, C], f32, kind="ExternalOutput")

    OWN = (3, 7, 11, 15)  # own position blocks (slot s -> position 4s+3)

    with tile.TileContext(nc) as tc:
        with (
            tc.tile_pool(name="const", bufs=1) as constp,
            tc.tile_pool(name="keep", bufs=1) as keepp,
            tc.tile_pool(name="w2pool", bufs=1) as w2p,
        ):
            validv = constp.tile([128, NT], f32)
            nc.sync.dma_start(validv[:], validv_d[:])
            vones = constp.tile([128, NT], f32)
            nc.sync.dma_start(vones[:], vones_d[:])
            tri = constp.tile([128, 128], bf16)
            nc.sync.dma_start(tri[:], tri_d[:])
            ident = constp.tile([128, 128], bf16)
            nc.sync.dma_start(ident[:], ident_d[:])
            bqs = constp.tile([128, HP], f32)
            nc.sync.dma_start(bqs[:], bq_d[:])
            bks = constp.tile([128, HP], f32)
            nc.sync.dma_start(bks[:], bk_d[:])
            b1s = constp.tile([128, NF], f32)
            nc.sync.dma_start(b1s[:], b1_d[:])
            b2row = constp.tile([1, C], bf16)
            nc.sync.dma_start(b2row[:], b2_d[:])
            g1b = constp.tile([128, C], bf16)
            nc.sync.dma_start(g1b[:], g1b_d[:])
            b1rb = constp.tile([128, C], f32)
            nc.sync.dma_start(b1rb[:], b1rb_d[:])
            ones1 = constp.tile([1, 128], bf16)
            nc.vector.memset(ones1[:], 1.0)
            onesf = constp.tile([128, H, 1], f32)
            nc.vector.memset(onesf[:], 1.0)
            eps_t = constp.tile([128, 1], f32)
            nc.vector.memset(eps_t[:], EPS)

            # w2 (fp8, small) arrives early so its DMA overlaps everything
            w2 = w2p.tile([128, JF, 2, C], fp8, name="w28")
            nc.sync.dma_start(w2[:], w2_d[:])

            xn_keep = keepp.tile([128, 4, C], f32)   # own rows (slot order), fp32
            x1 = keepp.tile([128, 4, C], f32)        # post-attention residual
            y_sb = keepp.tile([128, 4, H, 65], bf16)  # y token-major; k=3-s order

            with (
                tc.tile_pool(name="xnT8", bufs=1) as xnT8p,
                tc.tile_pool(name="QT", bufs=1) as QTp,
                tc.tile_pool(name="KT", bufs=1) as KTp,
                tc.tile_pool(name="V", bufs=1) as Vp,
            ):
                xnT8 = xnT8p.tile([128, JC, 2, T], fp8)       # xn^T * 16
                QT = QTp.tile([128, HP, T], bf16)             # q (true scale)
                KT = KTp.tile([128, HP, R], bf16)             # k own, col k=3-s
                xnT8own = KTp.tile([128, JC, 2, R], fp8)
                Vt = Vp.tile([128, NT, H, VS], bf16)          # v true, [..,64]=1

                # ===== Phase A+B: LN1, transpose, QKV (interleaved) =====
                with (
                    tc.tile_pool(name="xin", bufs=3) as xinp,
                    tc.tile_pool(name="stat", bufs=4) as statp,
                    tc.tile_pool(name="xnbf", bufs=3) as xnbfp,
                    tc.tile_pool(name="wqkv", bufs=1) as wp,
                    tc.tile_pool(name="psT", bufs=2, space="PSUM") as psT,
                    tc.tile_pool(name="psQ", bufs=2, space="PSUM") as psQ,
                    tc.tile_pool(name="psV", bufs=2, space="PSUM") as psV,
                ):
                    wq = wp.tile([128, JC, 2, C], fp8, name="wq8")
                    nc.sync.dma_start(wq[:], wq_d[:])
                    wk = wp.tile([128, JC, 2, C], fp8, name="wk8")
                    nc.sync.dma_start(wk[:], wk_d[:])
                    wv = wp.tile([128, JC, 2, C], fp8, name="wv8")
                    nc.sync.dma_start(wv[:], wv_d[:])

                    TILE_ORDER = (3, 7, 11, 15, 0, 1, 2, 4, 5, 6,
                                  8, 9, 10, 12, 13, 14)
                    qdone = [False] * 4
                    done = set()

                    def emit_front(tb):
                        # DMA + LN stats + normalize (vector/ACT, no PSUM)
                        # (emitted inline, in tile order)
                        xt = xinp.tile([128, C], f32, tag="xt", name="xt")
                        nc.sync.dma_start(xt[:], x_ctx[tb * 128:(tb + 1) * 128, :])
                        st6 = statp.tile([128, 2, 6], f32, tag="st6", name="st6")
                        for g in range(2):
                            nc.vector.bn_stats(
                                st6[:, g, :], xt[:, g * 384:(g + 1) * 384]
                            )
                        st2 = statp.tile([128, 2], f32, tag="st2", name="st2")
                        nc.vector.bn_aggr(st2[:], st6[:])
                        std = statp.tile([128, 1], f32, tag="std", name="std")
                        nc.scalar.activation(std[:], st2[:, 1:2], AF.Sqrt, bias=eps_t[:])
                        rstd = statp.tile([128, 1], f32, tag="rstd", name="rstd")
                        nc.vector.reciprocal(rstd[:], std[:])
                        nmb = statp.tile([128, 1], f32, tag="nmb", name="nmb")
                        nc.vector.tensor_scalar(
                            nmb[:], st2[:, 0:1], rstd[:], -1.0,
                            op0=ALU.mult, op1=ALU.mult,
                        )
                        xn_bf = xnbfp.tile([128, C], bf16, tag="xn_bf", name="xn_bf")
                        nc.scalar.activation(
                            xn_bf[:], xt[:], AF.Identity, bias=nmb[:], scale=rstd[:]
                        )
                        if tb in OWN:
                            s = OWN.index(tb)
                            nc.vector.tensor_scalar(
                                xn_keep[:, s, :], xt[:], st2[:, 0:1], rstd[:],
                                op0=ALU.subtract, op1=ALU.mult,
                            )
                        return xn_bf

                    def emit_back(tb, xn_bf):
                        done.add(tb)
                        tp = psT.tile([128, JC, 2, 128], bf16, tag="psT", name="tp")
                        for cb in range(NC):
                            nc.tensor.matmul(
                                tp[:, cb // 2, cb % 2, :],
                                xn_bf[:, cb * 128:(cb + 1) * 128],
                                ident[:], is_transpose=True, start=True, stop=True,
                            )
                        if tb % 2 == 0:
                            nc.vector.tensor_scalar(
                                xnT8[:, :, :, tb * 128:(tb + 1) * 128], tp[:],
                                SX, None, op0=ALU.mult,
                            )
                        else:
                            nc.scalar.mul(
                                xnT8[:, :, :, tb * 128:(tb + 1) * 128], tp[:], SX
                            )

                        # V projection for this tile (fp8 DoubleRow)
                        for g in range(2):
                            ps = psV.tile([128, 6, 64], f32, tag="psV", name="psv")
                            for j in range(JC):
                                nc.tensor.matmul(
                                    ps[:], xnT8[:, j, :, tb * 128:(tb + 1) * 128],
                                    wv[:, j, :, g * 384:(g + 1) * 384],
                                    start=(j == 0), stop=(j == JC - 1),
                                    perf_mode=PM.DoubleRow,
                                )
                            if (tb + g) % 2 == 0:
                                nc.vector.tensor_scalar(
                                    Vt[:, tb, g * 6:(g + 1) * 6, 0:64],
                                    ps[:], validv[:, tb:tb + 1], None, op0=ALU.mult,
                                )
                            else:
                                nc.scalar.activation(
                                    Vt[:, tb, g * 6:(g + 1) * 6, 0:64], ps[:],
                                    AF.Identity, scale=validv[:, tb:tb + 1],
                                )
                        nc.vector.tensor_scalar(
                            Vt[:, tb, :, 64:65], onesf[:],
                            vones[:, tb:tb + 1], None, op0=ALU.mult,
                        )
                        if tb in OWN:
                            s = OWN.index(tb)
                            nc.gpsimd.tensor_copy(
                                xnT8own[:, :, :, (3 - s) * 128:(4 - s) * 128],
                                xnT8[:, :, :, tb * 128:(tb + 1) * 128],
                            )
                        if all(p in done for p in OWN) and not qdone[0] \
                                and tb == 15:
                            qdone[0] = True
                            for hp in range(HP):
                                ps = psQ.tile([128, 512], f32, tag="psQ", name="psk")
                                for j in range(JC):
                                    nc.tensor.matmul(
                                        ps[:], wk[:, j, :, hp * 128:(hp + 1) * 128],
                                        xnT8own[:, j, :, :],
                                        start=(j == 0), stop=(j == JC - 1),
                                        perf_mode=PM.DoubleRow,
                                    )
                                nc.vector.tensor_scalar(
                                    KT[:, hp, :], ps[:], 1.0 / SXW, bks[:, hp:hp + 1],
                                    op0=ALU.mult, op1=ALU.add,
                                )
                        for nb in range(4):
                            grp = {4 * nb, 4 * nb + 1, 4 * nb + 2, 4 * nb + 3}
                            key = "q%d" % nb
                            if not (grp <= done) or key in done:
                                continue
                            done.add(key)
                            for hp in range(HP):
                                ps = psQ.tile([128, 512], f32, tag="psQ", name="psq")
                                for j in range(JC):
                                    nc.tensor.matmul(
                                        ps[:], wq[:, j, :, hp * 128:(hp + 1) * 128],
                                        xnT8[:, j, :, nb * 512:(nb + 1) * 512],
                                        start=(j == 0), stop=(j == JC - 1),
                                        perf_mode=PM.DoubleRow,
                                    )
                                if hp % 2 == 0:
                                    nc.scalar.activation(
                                        QT[:, hp, nb * 512:(nb + 1) * 512], ps[:],
                                        AF.Identity, bias=bqs[:, hp:hp + 1],
                                        scale=1.0 / SXW,
                                    )
                                else:
                                    nc.vector.tensor_scalar(
                                        QT[:, hp, nb * 512:(nb + 1) * 512], ps[:],
                                        1.0 / SXW, bqs[:, hp:hp + 1],
                                        op0=ALU.mult, op1=ALU.add,
                                    )

                    for tb in TILE_ORDER:
                        emit_back(tb, emit_front(tb))

                # ===== Phase C: attention =====
                with (
                    tc.tile_pool(name="exps", bufs=2) as expp,
                    tc.tile_pool(name="yT", bufs=2) as ytp,
                    tc.tile_pool(name="psS", bufs=3, space="PSUM") as psS,
                    tc.tile_pool(name="psY", bufs=1, space="PSUM") as psY,
                ):
                    def emit_scores(h, expST):
                        # generator: one step per ctx pair (scores + exp)
                        hp, off = h // 2, 64 * (h % 2)
                        for jp in range(NT // 2):
                            Np = (4 - jp // 2) * 128
                            ps = psS.tile([128, 2, 512], f32, tag="psS", name="pss")
                            for ql in range(2):
                                P = 2 * jp + ql
                                diag = (P % 4 == 3)
                                nc.tensor.matmul(
                                    ps[:, ql, 0:Np],
                                    QT[off:off + 64, hp, P * 128:(P + 1) * 128],
                                    KT[off:off + 64, hp, 0:Np],
                                    start=True, stop=not diag,
                                    skip_group_check=diag,
                                )
                                if diag:
                                    nc.tensor.matmul(
                                        ps[:, ql, Np - 128:Np],
                                        ident[:], tri[:],
                                        start=False, stop=True,
                                        skip_group_check=True,
                                    )
                            nc.scalar.activation(
                                expST[:, 2 * jp:2 * jp + 2, 0:Np], ps[:, :, 0:Np],
                                AF.Exp, scale=0.125,
                            )
                            yield

                    def emit_pv(h, expST):
                        # generator: one step per ctx pair (2 PV matmuls)
                        psy = psY.tile([128, 512], f32, tag="psY", name="psy")
                        for jp in range(NT // 2):
                            for ql in range(2):
                                P = 2 * jp + ql
                                Np = (4 - P // 4) * 128
                                nc.tensor.matmul(
                                    psy[0:65, 0:Np],
                                    Vt[:, P, h, 0:65],
                                    expST[:, P, 0:Np],
                                    start=(P == 0), stop=(P == NT - 1),
                                    skip_group_check=True,
                                )
                            yield
                        yTb = ytp.tile([128, 512], bf16, tag="yT", name="yT")
                        if h % 2 == 0:
                            nc.vector.tensor_copy(yTb[0:65, :], psy[0:65, :])
                        else:
                            nc.scalar.copy(yTb[0:65, :], psy[0:65, :])
                        tpy = psY.tile([128, 4, 66], bf16, tag="psTy", name="tpy")
                        for k in range(4):
                            nc.tensor.matmul(
                                tpy[:, k, 0:65], yTb[0:65, k * 128:(k + 1) * 128],
                                ident[0:65, 0:65], is_transpose=True,
                                start=True, stop=True,
                            )
                        if h % 2 == 0:
                            nc.scalar.copy(y_sb[:, :, h, :], tpy[:, :, 0:65])
                        else:
                            nc.vector.tensor_copy(y_sb[:, :, h, :], tpy[:, :, 0:65])
                        yield

                    # software pipeline: head h scores/exp woven with h-1's PV
                    exp_tiles = {}
                    prev_pv = None
                    for h in range(H):
                        exp_tiles[h] = expp.tile([128, NT, 512], bf16,
                                                 tag="expST", name="expST")
                        sc = emit_scores(h, exp_tiles[h])
                        for _ in sc:
                            if prev_pv is not None:
                                next(prev_pv, None)
                        if prev_pv is not None:
                            for _ in prev_pv:  # drain tail (yTb/y_sb copies)
                                pass
                        prev_pv = emit_pv(h, exp_tiles[h])
                    for _ in prev_pv:
                        pass

            # ===== Phase D/E/F: y-norm + residual, LN2, MLP =====
            with (
                tc.tile_pool(name="w1pool", bufs=1) as w1p,
                tc.tile_pool(name="x1nT", bufs=1) as x1nTp,
                tc.tile_pool(name="h1T8", bufs=1) as h1p,
                tc.tile_pool(name="ynorm", bufs=2) as ynp,
                tc.tile_pool(name="stat2", bufs=4) as stat2p,
                tc.tile_pool(name="x1nbf", bufs=2) as x1nbfp,
                tc.tile_pool(name="psT2", bufs=2, space="PSUM") as psT2,
            ):
                w1 = w1p.tile([128, NC, F], bf16, name="w1b")
                nc.sync.dma_start(w1[:], w1_d[:])
                x1nT = x1nTp.tile([128, NC, R], bf16)
                h1T8 = h1p.tile([128, JF, 2, R], fp8)

                for s in range(4):
                    k = 3 - s
                    den = ynp.tile([128, H], f32, tag="den", name="den")
                    nc.vector.tensor_copy(den[:], y_sb[:, k, :, 64])
                    rec = ynp.tile([128, H], f32, tag="rec", name="rec")
                    nc.vector.reciprocal(rec[:], den[:])
                    yf = ynp.tile([128, H, DH], f32, tag="yf", name="yf")
                    for hh in range(H):
                        if hh % 2 == 0:
                            nc.vector.tensor_scalar(
                                yf[:, hh, :], y_sb[:, k, hh, 0:64],
                                rec[:, hh:hh + 1], None, op0=ALU.mult,
                            )
                        else:
                            nc.scalar.activation(
                                yf[:, hh, :], y_sb[:, k, hh, 0:64],
                                AF.Identity, scale=rec[:, hh:hh + 1],
                            )
                    nc.vector.tensor_mul(x1[:, s, :], xn_keep[:, s, :], g1b[:])
                    nc.vector.tensor_add(x1[:, s, :], x1[:, s, :], b1rb[:])
                    nc.vector.tensor_add(
                        x1[:, s, :], x1[:, s, :],
                        yf[:].rearrange("p h d -> p (h d)"),
                    )
                    # LN2 for this slot
                    st6 = stat2p.tile([128, 2, 6], f32, tag="st6", name="st6b")
                    for g in range(2):
                        nc.vector.bn_stats(
                            st6[:, g, :], x1[:, s, g * 384:(g + 1) * 384]
                        )
                    st2 = stat2p.tile([128, 2], f32, tag="st2", name="st2b")
                    nc.vector.bn_aggr(st2[:], st6[:])
                    std = stat2p.tile([128, 1], f32, tag="std", name="stdb")
                    nc.scalar.activation(std[:], st2[:, 1:2], AF.Sqrt, bias=eps_t[:])
                    rstd = stat2p.tile([128, 1], f32, tag="rstd", name="rstdb")
                    nc.vector.reciprocal(rstd[:], std[:])
                    nmb = stat2p.tile([128, 1], f32, tag="nmb", name="nmbb")
                    nc.vector.tensor_scalar(
                        nmb[:], st2[:, 0:1], rstd[:], -1.0,
                        op0=ALU.mult, op1=ALU.mult,
                    )
                    x1n = x1nbfp.tile([128, C], bf16, tag="x1n", name="x1n")
                    nc.scalar.activation(
                        x1n[:], x1[:, s, :], AF.Identity, bias=nmb[:], scale=rstd[:]
                    )
                    tp = psT2.tile([128, NC, 128], bf16, tag="psT2", name="tpb")
                    for cb in range(NC):
                        nc.tensor.matmul(
                            tp[:, cb, :],
                            x1n[:, cb * 128:(cb + 1) * 128],
                            ident[:], is_transpose=True, start=True, stop=True,
                        )
                    if s % 2 == 0:
                        nc.vector.tensor_copy(
                            x1nT[:, :, s * 128:(s + 1) * 128], tp[:]
                        )
                    else:
                        nc.scalar.copy(x1nT[:, :, s * 128:(s + 1) * 128], tp[:])

                with (
                    tc.tile_pool(name="psH", bufs=2, space="PSUM") as psH,
                    tc.tile_pool(name="psO", bufs=2, space="PSUM") as psO,
                    tc.tile_pool(name="outp", bufs=2) as outp,
                ):
                    for nf in range(NF):
                        ps = psH.tile([128, 512], f32, tag="psH", name="psh")
                        for cb in range(NC):
                            nc.tensor.matmul(
                                ps[:], w1[:, cb, nf * 128:(nf + 1) * 128],
                                x1nT[:, cb, :],
                                start=(cb == 0), stop=(cb == NC - 1),
                            )
                        nc.scalar.activation(
                            h1T8[:, nf // 2, nf % 2, :], ps[:],
                            AF.Gelu, bias=b1s[:, nf:nf + 1],
                        )
                    for s in range(4):
                        o_sb = outp.tile([128, C], f32, tag="o", name="o_sb")
                        for g in range(2):
                            ps = psO.tile([128, 384], f32, tag="psO", name="pso")
                            for jf in range(JF):
                                nc.tensor.matmul(
                                    ps[:],
                                    h1T8[:, jf, :, s * 128:(s + 1) * 128],
                                    w2[:, jf, :, g * 384:(g + 1) * 384],
                                    start=(jf == 0), stop=False,
                                    perf_mode=PM.DoubleRow,
                                    skip_group_check=True,
                                )
                            nc.tensor.matmul(
                                ps[:], ones1[:], b2row[:, g * 384:(g + 1) * 384],
                                start=False, stop=True, skip_group_check=True,
                            )
                            nc.vector.scalar_tensor_tensor(
                                o_sb[:, g * 384:(g + 1) * 384], ps[:], 1.0 / SW,
                                x1[:, s, g * 384:(g + 1) * 384],
                                op0=ALU.mult, op1=ALU.add,
                            )
                        nc.sync.dma_start(out_d[s * 128:(s + 1) * 128, :], o_sb[:])

    nc.compile()
    return nc


def _prep_shared(inputs):
    import ml_dtypes

    f = np.float32
    bf = ml_dtypes.bfloat16
    f8 = ml_dtypes.float8_e4m3
    g1 = np.asarray(inputs["ln1_g"], f)
    b1r = np.asarray(inputs["ln1_b"], f)
    g2 = np.asarray(inputs["ln2_g"], f)
    b2r = np.asarray(inputs["ln2_b"], f)
    Wq, Wk, Wv = (np.asarray(inputs[k], f) for k in ("Wq", "Wk", "Wv"))
    W1, W2 = np.asarray(inputs["W1"], f), np.asarray(inputs["W2"], f)

    def dr_pack(w, scale):
        # [K, M] -> [128, K/256, 2, M] with channel k = j*256 + q*128 + p
        K, M = w.shape
        return np.ascontiguousarray(
            (w * scale).reshape(K // 256, 2, 128, M).transpose(2, 0, 1, 3)
        ).astype(f8)

    def bf_pack(w):
        # [K, M] -> [128, K/128, M]
        K, M = w.shape
        return np.ascontiguousarray(
            w.reshape(K // 128, 128, M).transpose(1, 0, 2)
        ).astype(bf)

    def colmajor_bias(b, n):
        return np.ascontiguousarray(b.reshape(n, 128).T)

    bv_eff = b1r @ Wv + np.asarray(inputs["bv"], f)
    rows = np.arange(128)
    trimask = np.where(rows[:, None] > rows[None, :], -1e5, 0.0).astype(bf)

    return {
        "wq8": dr_pack(g1[:, None] * Wq, SW),
        "wk8": dr_pack(g1[:, None] * Wk, SW),
        "wv8": dr_pack(g1[:, None] * Wv, SW),
        "w1b": bf_pack(g2[:, None] * W1),
        "w28": dr_pack(W2, SW),
        "bq": colmajor_bias(b1r @ Wq + np.asarray(inputs["bq"], f), HP),
        "bk": colmajor_bias(b1r @ Wk + np.asarray(inputs["bk"], f), HP),
        "b1": colmajor_bias(b2r @ W1 + np.asarray(inputs["b1"], f), NF),
        "b2row": np.ascontiguousarray(np.asarray(inputs["b2"], f)[None, :]).astype(bf),
        "g1b": np.ascontiguousarray(np.broadcast_to(g1, (128, C))).astype(bf),
        "b1rb": np.ascontiguousarray(np.broadcast_to(b1r + bv_eff, (128, C))).astype(f),
        "trimask": np.ascontiguousarray(trimask),
        "identb": np.eye(128, dtype=f).astype(bf),
    }


def kernel(**inputs):
    from concourse.bass_utils import run_bass_kernel_spmd

    if "nc" not in _CACHE:
        _CACHE["nc"] = _build_program()
    nc = _CACHE["nc"]

    x = np.asarray(inputs["x"], np.float32)
    shared = _prep_shared(inputs)

    in_maps = []
    for c8 in range(8):
        b, c = c8 // 4, c8 % 4
        pad = 3 - c
        x_ctx = np.zeros((T, C), np.float32)
        x_ctx[pad * 128:] = x[b, 0:(13 + c) * 128]
        valid = np.zeros(NT, np.float32)
        valid[pad:] = 1.0
        m = dict(shared)
        m["x_ctx"] = x_ctx
        m["validv"] = np.ascontiguousarray(
            np.broadcast_to(valid * (1.0 / SXW), (128, NT)).astype(np.float32))
        m["vones"] = np.ascontiguousarray(
            np.broadcast_to(valid, (128, NT)).astype(np.float32))
        in_maps.append(m)

    trace = bool(int(os.environ.get("KERNEL_TRACE", "0")))
    try:
        res = run_bass_kernel_spmd(nc, in_maps, core_ids=list(range(8)), trace=trace)
    except ModuleNotFoundError:
        res = run_bass_kernel_spmd(nc, in_maps, core_ids=list(range(8)), trace=False)
    _CACHE["last_result"] = res

    out = np.empty((B, T, C), np.float32)
    for c8 in range(8):
        b, c = c8 // 4, c8 % 4
        for s in range(4):
            blk = c + 4 * s
            out[b, blk * 128:(blk + 1) * 128] = \
                res.results[c8]["out"][s * 128:(s + 1) * 128]
    return out


# revision 14
# speedup vs baseline: 1.2317x; 1.0295x over previous
"""Trainium2 Bass kernel for a GPT-style decoder block (B=2, T=2048, C=768, H=12).

Sharding: 8 cores = 2 batches x 4 interleaved block-sets. Core c owns 128-row
blocks {c, c+4, c+8, c+12} of its batch. Its context buffer holds the 16
position-blocks [zeros x (3-c) | blocks 0..12+c]; the own blocks then sit at
the STATIC positions {3, 7, 11, 15} with causal context = position prefixes of
length {4, 8, 12, 16} blocks. Every core therefore runs the same instruction
stream while doing the load-balanced share (40/64) of the causal attention
work; the inserted zero blocks are masked via a per-block validity scale on V
(and on the denominator ones-column), so they contribute exactly 0 to both the
attention numerator and the softmax denominator.

Numerics: Q/K/V projections and the second MLP matmul run in fp8e4 with
DoubleRow perf mode (two 128-channel contraction chunks per instruction);
scores, P, V and the first MLP matmul stay bf16 (fp8 everywhere pushes the
fixed-seed rel-err past the 2e-2 gate; this mix measures ~1.6e-2 in numpy).
LN statistics, softmax normalization, residuals and the output stay fp32.
fp8 scales (weights x512, activations x16) are divided out on PSUM->SBUF
copies.

Note: reference computes scores = K @ Q^T, so the output-row operand is K (own
rows) and the context operand is Q/V. The causal triangle on each own block's
diagonal position is applied by accumulating a -1e5 upper-triangle mask into
the scores PSUM with one extra bf16 matmul. V / Q projections are interleaved
into the LN1 loop so TensorE stays busy during the per-tile LN chains.
"""

import os

import numpy as np

B, T, C = 2, 2048, 768
H, DH = 12, 64
F = 4 * C
R = 512            # own rows per core
NT = 16            # ctx position blocks
NC = C // 128      # 6
JC = NC // 2       # 3 channel pairs
NF = F // 128      # 24
JF = NF // 2       # 12 hidden pairs
HP = H // 2        # 6 head pairs
VS = 66            # per-head stride in Vt (64 v + 1 ones + pad)
EPS = 1e-3
SX = 16.0          # fp8 scale on normalized activations
SW = 512.0         # fp8 scale on weights
SXW = SX * SW      # 8192

_CACHE = {}


def _build_program():
    import concourse.bass as bass  # noqa: F401
    import concourse.mybir as mybir
    import concourse.tile as tile
    from concourse import bacc

    dt = mybir.dt
    f32 = dt.float32
    bf16 = dt.bfloat16
    fp8 = dt.float8e4
    AF = mybir.ActivationFunctionType
    ALU = mybir.AluOpType
    PM = mybir.MatmulPerfMode

    nc = bacc.Bacc("TRN2", target_bir_lowering=False, debug=False, num_devices=8)

    # ---- DRAM I/O ----
    x_ctx = nc.dram_tensor("x_ctx", [T, C], f32, kind="ExternalInput")
    validv_d = nc.dram_tensor("validv", [128, NT], f32, kind="ExternalInput")
    vones_d = nc.dram_tensor("vones", [128, NT], f32, kind="ExternalInput")
    wq_d = nc.dram_tensor("wq8", [128, JC, 2, C], fp8, kind="ExternalInput")
    wk_d = nc.dram_tensor("wk8", [128, JC, 2, C], fp8, kind="ExternalInput")
    wv_d = nc.dram_tensor("wv8", [128, JC, 2, C], fp8, kind="ExternalInput")
    w1_d = nc.dram_tensor("w1b", [128, NC, F], bf16, kind="ExternalInput")
    w2_d = nc.dram_tensor("w28", [128, JF, 2, C], fp8, kind="ExternalInput")
    bq_d = nc.dram_tensor("bq", [128, HP], f32, kind="ExternalInput")
    bk_d = nc.dram_tensor("bk", [128, HP], f32, kind="ExternalInput")
    b1_d = nc.dram_tensor("b1", [128, NF], f32, kind="ExternalInput")
    b2_d = nc.dram_tensor("b2row", [1, C], bf16, kind="ExternalInput")
    g1b_d = nc.dram_tensor("g1b", [128, C], bf16, kind="ExternalInput")
    b1rb_d = nc.dram_tensor("b1rb", [128, C], f32, kind="ExternalInput")
    tri_d = nc.dram_tensor("trimask", [128, 128], bf16, kind="ExternalInput")
    ident_d = nc.dram_tensor("identb", [128, 128], bf16, kind="ExternalInput")
    out_d = nc.dram_tensor("out", [R, C], f32, kind="ExternalOutput")

    OWN = (3, 7, 11, 15)  # own position blocks (slot s -> position 4s+3)

    with tile.TileContext(nc) as tc:
        with (
            tc.tile_pool(name="const", bufs=1) as constp,
            tc.tile_pool(name="keep", bufs=1) as keepp,
            tc.tile_pool(name="w2pool", bufs=1) as w2p,
        ):
            validv = constp.tile([128, NT], f32)
            nc.sync.dma_start(validv[:], validv_d[:])
            vones = constp.tile([128, NT], f32)
            nc.sync.dma_start(vones[:], vones_d[:])
            tri = constp.tile([128, 128], bf16)
            nc.sync.dma_start(tri[:], tri_d[:])
            ident = constp.tile([128, 128], bf16)
            nc.sync.dma_start(ident[:], ident_d[:])
            bqs = constp.tile([128, HP], f32)
            nc.sync.dma_start(bqs[:], bq_d[:])
            bks = constp.tile([128, HP], f32)
            nc.sync.dma_start(bks[:], bk_d[:])
            b1s = constp.tile([128, NF], f32)
            nc.sync.dma_start(b1s[:], b1_d[:])
            b2row = constp.tile([1, C], bf16)
            nc.sync.dma_start(b2row[:], b2_d[:])
            g1b = constp.tile([128, C], bf16)
            nc.sync.dma_start(g1b[:], g1b_d[:])
            b1rb = constp.tile([128, C], f32)
            nc.sync.dma_start(b1rb[:], b1rb_d[:])
            ones1 = constp.tile([1, 128], bf16)
            nc.vector.memset(ones1[:], 1.0)
            onesf = constp.tile([128, H, 1], f32)
            nc.vector.memset(onesf[:], 1.0)
            eps_t = constp.tile([128, 1], f32)
            nc.vector.memset(eps_t[:], EPS)

            # w2 (fp8, small) arrives early so its DMA overlaps everything
            w2 = w2p.tile([128, JF, 2, C], fp8, name="w28")
            nc.sync.dma_start(w2[:], w2_d[:])

            xn_keep = keepp.tile([128, 4, C], f32)   # own rows (slot order), fp32
            x1 = keepp.tile([128, 4, C], f32)        # post-attention residual
            y_sb = keepp.tile([128, 4, H, 65], bf16)  # y token-major; k=3-s order

            with (
                tc.tile_pool(name="xnT8", bufs=1) as xnT8p,
                tc.tile_pool(name="QT", bufs=1) as QTp,
                tc.tile_pool(name="KT", bufs=1) as KTp,
                tc.tile_pool(name="V", bufs=1) as Vp,
            ):
                xnT8 = xnT8p.tile([128, JC, 2, T], fp8)       # xn^T * 16
                QT = QTp.tile([128, HP, T], bf16)             # q (true scale)
                KT = KTp.tile([128, HP, R], bf16)             # k own, col k=3-s
                xnT8own = KTp.tile([128, JC, 2, R], fp8)
                Vt = Vp.tile([128, NT, H, VS], bf16)          # v true, [..,64]=1

                # ===== Phase A+B: LN1, transpose, QKV (interleaved) =====
                with (
                    tc.tile_pool(name="xin", bufs=3) as xinp,
                    tc.tile_pool(name="stat", bufs=4) as statp,
                    tc.tile_pool(name="xnbf", bufs=3) as xnbfp,
                    tc.tile_pool(name="wqkv", bufs=1) as wp,
                    tc.tile_pool(name="psT", bufs=2, space="PSUM") as psT,
                    tc.tile_pool(name="psQ", bufs=2, space="PSUM") as psQ,
                    tc.tile_pool(name="psV", bufs=2, space="PSUM") as psV,
                ):
                    wq = wp.tile([128, JC, 2, C], fp8, name="wq8")
                    nc.sync.dma_start(wq[:], wq_d[:])
                    wk = wp.tile([128, JC, 2, C], fp8, name="wk8")
                    nc.sync.dma_start(wk[:], wk_d[:])
                    wv = wp.tile([128, JC, 2, C], fp8, name="wv8")
                    nc.sync.dma_start(wv[:], wv_d[:])

                    TILE_ORDER = (3, 7, 11, 15, 0, 1, 2, 4, 5, 6,
                                  8, 9, 10, 12, 13, 14)
                    qdone = [False] * 4
                    done = set()

                    def emit_front(tb):
                        # DMA + LN stats + normalize (vector/ACT, no PSUM)
                        # (emitted inline, in tile order)
                        xt = xinp.tile([128, C], f32, tag="xt", name="xt")
                        nc.sync.dma_start(xt[:], x_ctx[tb * 128:(tb + 1) * 128, :])
                        st6 = statp.tile([128, 2, 6], f32, tag="st6", name="st6")
                        for g in range(2):
                            nc.vector.bn_stats(
                                st6[:, g, :], xt[:, g * 384:(g + 1) * 384]
                            )
                        st2 = statp.tile([128, 2], f32, tag="st2", name="st2")
                        nc.vector.bn_aggr(st2[:], st6[:])
                        std = statp.tile([128, 1], f32, tag="std", name="std")
                        nc.scalar.activation(std[:], st2[:, 1:2], AF.Sqrt, bias=eps_t[:])
                        rstd = statp.tile([128, 1], f32, tag="rstd", name="rstd")
                        nc.vector.reciprocal(rstd[:], std[:])
                        nmb = statp.tile([128, 1], f32, tag="nmb", name="nmb")
                        nc.vector.tensor_scalar(
                            nmb[:], st2[:, 0:1], rstd[:], -1.0,
                            op0=ALU.mult, op1=ALU.mult,
                        )
                        xn_bf = xnbfp.tile([128, C], bf16, tag="xn_bf", name="xn_bf")
                        nc.scalar.activation(
                            xn_bf[:], xt[:], AF.Identity, bias=nmb[:], scale=rstd[:]
                        )
                        if tb in OWN:
                            s = OWN.index(tb)
                            nc.vector.tensor_scalar(
                                xn_keep[:, s, :], xt[:], st2[:, 0:1], rstd[:],
                                op0=ALU.subtract, op1=ALU.mult,
                            )
                        return xn_bf

                    def emit_back(tb, xn_bf):
                        done.add(tb)
                        tp = psT.tile([128, JC, 2, 128], bf16, tag="psT", name="tp")
                        for cb in range(NC):
                            nc.tensor.matmul(
                                tp[:, cb // 2, cb % 2, :],
                                xn_bf[:, cb * 128:(cb + 1) * 128],
                                ident[:], is_transpose=True, start=True, stop=True,
                            )
                        if tb % 2 == 0:
                            nc.vector.tensor_scalar(
                                xnT8[:, :, :, tb * 128:(tb + 1) * 128], tp[:],
                                SX, None, op0=ALU.mult,
                            )
                        else:
                            nc.scalar.mul(
                                xnT8[:, :, :, tb * 128:(tb + 1) * 128], tp[:], SX
                            )

                        # V projection for this tile (fp8 DoubleRow)
                        for g in range(2):
                            ps = psV.tile([128, 6, 64], f32, tag="psV", name="psv")
                            for j in range(JC):
                                nc.tensor.matmul(
                                    ps[:], xnT8[:, j, :, tb * 128:(tb + 1) * 128],
                                    wv[:, j, :, g * 384:(g + 1) * 384],
                                    start=(j == 0), stop=(j == JC - 1),
                                    perf_mode=PM.DoubleRow,
                                )
                            if (tb + g) % 2 == 0:
                                nc.vector.tensor_scalar(
                                    Vt[:, tb, g * 6:(g + 1) * 6, 0:64],
                                    ps[:], validv[:, tb:tb + 1], None, op0=ALU.mult,
                                )
                            else:
                                nc.scalar.activation(
                                    Vt[:, tb, g * 6:(g + 1) * 6, 0:64], ps[:],
                                    AF.Identity, scale=validv[:, tb:tb + 1],
                                )
                        nc.vector.tensor_scalar(
                            Vt[:, tb, :, 64:65], onesf[:],
                            vones[:, tb:tb + 1], None, op0=ALU.mult,
                        )
                        if tb in OWN:
                            s = OWN.index(tb)
                            nc.gpsimd.tensor_copy(
                                xnT8own[:, :, :, (3 - s) * 128:(4 - s) * 128],
                                xnT8[:, :, :, tb * 128:(tb + 1) * 128],
                            )
                        if all(p in done for p in OWN) and not qdone[0] \
                                and tb == 15:
                            qdone[0] = True
                            for hp in range(HP):
                                ps = psQ.tile([128, 512], f32, tag="psQ", name="psk")
                                for j in range(JC):
                                    nc.tensor.matmul(
                                        ps[:], wk[:, j, :, hp * 128:(hp + 1) * 128],
                                        xnT8own[:, j, :, :],
                                        start=(j == 0), stop=(j == JC - 1),
                                        perf_mode=PM.DoubleRow,
                                    )
                                nc.vector.tensor_scalar(
                                    KT[:, hp, :], ps[:], 1.0 / SXW, bks[:, hp:hp + 1],
                                    op0=ALU.mult, op1=ALU.add,
                                )
                        for nb in range(4):
                            grp = {4 * nb, 4 * nb + 1, 4 * nb + 2, 4 * nb + 3}
                            key = "q%d" % nb
                            if not (grp <= done) or key in done:
                                continue
                            done.add(key)
                            for hp in range(HP):
                                ps = psQ.tile([128, 512], f32, tag="psQ", name="psq")
                                for j in range(JC):
                                    nc.tensor.matmul(
                                        ps[:], wq[:, j, :, hp * 128:(hp + 1) * 128],
                                        xnT8[:, j, :, nb * 512:(nb + 1) * 512],
                                        start=(j == 0), stop=(j == JC - 1),
                                        perf_mode=PM.DoubleRow,
                                    )
                                if hp % 2 == 0:
                                    nc.scalar.activation(
                                        QT[:, hp, nb * 512:(nb + 1) * 512], ps[:],
                                        AF.Identity, bias=bqs[:, hp:hp + 1],
                                        scale=1.0 / SXW,
                                    )
                                else:
                                    nc.vector.tensor_scalar(
                                        QT[:, hp, nb * 512:(nb + 1) * 512], ps[:],
                                        1.0 / SXW, bqs[:, hp:hp + 1],
                                        op0=ALU.mult, op1=ALU.add,
                                    )

                    for tb in TILE_ORDER:
                        emit_back(tb, emit_front(tb))

                # ===== Phase C: attention =====
                with (
                    tc.tile_pool(name="exps", bufs=3) as expp,
                    tc.tile_pool(name="yT", bufs=2) as ytp,
                    tc.tile_pool(name="psS", bufs=3, space="PSUM") as psS,
                    tc.tile_pool(name="psY", bufs=1, space="PSUM") as psY,
                ):
                    def emit_scores(h, expST):
                        # generator: one step per ctx pair (scores + exp)
                        hp, off = h // 2, 64 * (h % 2)
                        for jp in range(NT // 2):
                            Np = (4 - jp // 2) * 128
                            ps = psS.tile([128, 2, 512], f32, tag="psS", name="pss")
                            for ql in range(2):
                                P = 2 * jp + ql
                                diag = (P % 4 == 3)
                                nc.tensor.matmul(
                                    ps[:, ql, 0:Np],
                                    QT[off:off + 64, hp, P * 128:(P + 1) * 128],
                                    KT[off:off + 64, hp, 0:Np],
                                    start=True, stop=not diag,
                                    skip_group_check=diag,
                                )
                                if diag:
                                    nc.tensor.matmul(
                                        ps[:, ql, Np - 128:Np],
                                        ident[:], tri[:],
                                        start=False, stop=True,
                                        skip_group_check=True,
                                    )
                            nc.scalar.activation(
                                expST[:, 2 * jp:2 * jp + 2, 0:Np], ps[:, :, 0:Np],
                                AF.Exp, scale=0.125,
                            )
                            yield

                    def emit_pv(h, expST):
                        # generator: one step per ctx pair (2 PV matmuls)
                        psy = psY.tile([128, 512], f32, tag="psY", name="psy")
                        for jp in range(NT // 2):
                            for ql in range(2):
                                P = 2 * jp + ql
                                Np = (4 - P // 4) * 128
                                nc.tensor.matmul(
                                    psy[0:65, 0:Np],
                                    Vt[:, P, h, 0:65],
                                    expST[:, P, 0:Np],
                                    start=(P == 0), stop=(P == NT - 1),
                                    skip_group_check=True,
                                )
                            yield
                        yTb = ytp.tile([128, 512], bf16, tag="yT", name="yT")
                        if h % 2 == 0:
                            nc.vector.tensor_copy(yTb[0:65, :], psy[0:65, :])
                        else:
                            nc.scalar.copy(yTb[0:65, :], psy[0:65, :])
                        tpy = psY.tile([128, 4, 66], bf16, tag="psTy", name="tpy")
                        for k in range(4):
                            nc.tensor.matmul(
                                tpy[:, k, 0:65], yTb[0:65, k * 128:(k + 1) * 128],
                                ident[0:65, 0:65], is_transpose=True,
                                start=True, stop=True,
                            )
                        if h % 2 == 0:
                            nc.scalar.copy(y_sb[:, :, h, :], tpy[:, :, 0:65])
                        else:
                            nc.vector.tensor_copy(y_sb[:, :, h, :], tpy[:, :, 0:65])
                        yield

                    # software pipeline: head h scores/exp woven with h-1's PV
                    exp_tiles = {}
                    prev_pv = None
                    for h in range(H):
                        exp_tiles[h] = expp.tile([128, NT, 512], bf16,
                                                 tag="expST", name="expST")
                        sc = emit_scores(h, exp_tiles[h])
                        for _ in sc:
                            if prev_pv is not None:
                                next(prev_pv, None)
                        if prev_pv is not None:
                            for _ in prev_pv:  # drain tail (yTb/y_sb copies)
                                pass
                        prev_pv = emit_pv(h, exp_tiles[h])
                    for _ in prev_pv:
                        pass

            # ===== Phase D/E/F: y-norm + residual, LN2, MLP =====
            with (
                tc.tile_pool(name="w1pool", bufs=1) as w1p,
                tc.tile_pool(name="x1nT", bufs=1) as x1nTp,
                tc.tile_pool(name="h1T8", bufs=1) as h1p,
                tc.tile_pool(name="ynorm", bufs=2) as ynp,
                tc.tile_pool(name="stat2", bufs=4) as stat2p,
                tc.tile_pool(name="x1nbf", bufs=2) as x1nbfp,
                tc.tile_pool(name="psT2", bufs=2, space="PSUM") as psT2,
            ):
                w1 = w1p.tile([128, NC, F], bf16, name="w1b")
                nc.sync.dma_start(w1[:], w1_d[:])
                x1nT = x1nTp.tile([128, NC, R], bf16)
                h1T8 = h1p.tile([128, JF, 2, R], fp8)

                for s in range(4):
                    k = 3 - s
                    den = ynp.tile([128, H], f32, tag="den", name="den")
                    nc.vector.tensor_copy(den[:], y_sb[:, k, :, 64])
                    rec = ynp.tile([128, H], f32, tag="rec", name="rec")
                    nc.vector.reciprocal(rec[:], den[:])
                    yf = ynp.tile([128, H, DH], f32, tag="yf", name="yf")
                    for hh in range(H):
                        if hh % 2 == 0:
                            nc.vector.tensor_scalar(
                                yf[:, hh, :], y_sb[:, k, hh, 0:64],
                                rec[:, hh:hh + 1], None, op0=ALU.mult,
                            )
                        else:
                            nc.scalar.activation(
                                yf[:, hh, :], y_sb[:, k, hh, 0:64],
                                AF.Identity, scale=rec[:, hh:hh + 1],
                            )
                    nc.vector.tensor_mul(x1[:, s, :], xn_keep[:, s, :], g1b[:])
                    nc.vector.tensor_add(x1[:, s, :], x1[:, s, :], b1rb[:])
                    nc.vector.tensor_add(
                        x1[:, s, :], x1[:, s, :],
                        yf[:].rearrange("p h d -> p (h d)"),
                    )
                    # LN2 for this slot
                    st6 = stat2p.tile([128, 2, 6], f32, tag="st6", name="st6b")
                    for g in range(2):
                        nc.vector.bn_stats(
                            st6[:, g, :], x1[:, s, g * 384:(g + 1) * 384]
                        )
                    st2 = stat2p.tile([128, 2], f32, tag="st2", name="st2b")
                    nc.vector.bn_aggr(st2[:], st6[:])
                    std = stat2p.tile([128, 1], f32, tag="std", name="stdb")
                    nc.scalar.activation(std[:], st2[:, 1:2], AF.Sqrt, bias=eps_t[:])
                    rstd = stat2p.tile([128, 1], f32, tag="rstd", name="rstdb")
                    nc.vector.reciprocal(rstd[:], std[:])
                    nmb = stat2p.tile([128, 1], f32, tag="nmb", name="nmbb")
                    nc.vector.tensor_scalar(
                        nmb[:], st2[:, 0:1], rstd[:], -1.0,
                        op0=ALU.mult, op1=ALU.mult,
                    )
                    x1n = x1nbfp.tile([128, C], bf16, tag="x1n", name="x1n")
                    nc.scalar.activation(
                        x1n[:], x1[:, s, :], AF.Identity, bias=nmb[:], scale=rstd[:]
                    )
                    tp = psT2.tile([128, NC, 128], bf16, tag="psT2", name="tpb")
                    for cb in range(NC):
                        nc.tensor.matmul(
                            tp[:, cb, :],
                            x1n[:, cb * 128:(cb + 1) * 128],
                            ident[:], is_transpose=True, start=True, stop=True,
                        )
                    if s % 2 == 0:
                        nc.vector.tensor_copy(
                            x1nT[:, :, s * 128:(s + 1) * 128], tp[:]
                        )
                    else:
                        nc.scalar.copy(x1nT[:, :, s * 128:(s + 1) * 128], tp[:])

                with (
                    tc.tile_pool(name="psH", bufs=2, space="PSUM") as psH,
                    tc.tile_pool(name="psO", bufs=2, space="PSUM") as psO,
                    tc.tile_pool(name="outp", bufs=2) as outp,
                ):
                    for nf in range(NF):
                        ps = psH.tile([128, 512], f32, tag="psH", name="psh")
                        for cb in range(NC):
                            nc.tensor.matmul(
                                ps[:], w1[:, cb, nf * 128:(nf + 1) * 128],
                                x1nT[:, cb, :],
                                start=(cb == 0), stop=(cb == NC - 1),
                            )
                        nc.scalar.activation(
                            h1T8[:, nf // 2, nf % 2, :], ps[:],
                            AF.Gelu, bias=b1s[:, nf:nf + 1],
                        )
                    for s in range(4):
                        o_sb = outp.tile([128, C], f32, tag="o", name="o_sb")
                        for g in range(2):
                            ps = psO.tile([128, 384], f32, tag="psO", name="pso")
                            for jf in range(JF):
                                nc.tensor.matmul(
                                    ps[:],
                                    h1T8[:, jf, :, s * 128:(s + 1) * 128],
                                    w2[:, jf, :, g * 384:(g + 1) * 384],
                                    start=(jf == 0), stop=False,
                                    perf_mode=PM.DoubleRow,
                                    skip_group_check=True,
                                )
                            nc.tensor.matmul(
                                ps[:], ones1[:], b2row[:, g * 384:(g + 1) * 384],
                                start=False, stop=True, skip_group_check=True,
                            )
                            nc.vector.scalar_tensor_tensor(
                                o_sb[:, g * 384:(g + 1) * 384], ps[:], 1.0 / SW,
                                x1[:, s, g * 384:(g + 1) * 384],
                                op0=ALU.mult, op1=ALU.add,
                            )
                        nc.sync.dma_start(out_d[s * 128:(s + 1) * 128, :], o_sb[:])

    nc.compile()
    return nc


def _prep_shared(inputs):
    import ml_dtypes

    f = np.float32
    bf = ml_dtypes.bfloat16
    f8 = ml_dtypes.float8_e4m3
    g1 = np.asarray(inputs["ln1_g"], f)
    b1r = np.asarray(inputs["ln1_b"], f)
    g2 = np.asarray(inputs["ln2_g"], f)
    b2r = np.asarray(inputs["ln2_b"], f)
    Wq, Wk, Wv = (np.asarray(inputs[k], f) for k in ("Wq", "Wk", "Wv"))
    W1, W2 = np.asarray(inputs["W1"], f), np.asarray(inputs["W2"], f)

    def dr_pack(w, scale):
        # [K, M] -> [128, K/256, 2, M] with channel k = j*256 + q*128 + p
        K, M = w.shape
        return np.ascontiguousarray(
            (w * scale).reshape(K // 256, 2, 128, M).transpose(2, 0, 1, 3)
        ).astype(f8)

    def bf_pack(w):
        # [K, M] -> [128, K/128, M]
        K, M = w.shape
        return np.ascontiguousarray(
            w.reshape(K // 128, 128, M).transpose(1, 0, 2)
        ).astype(bf)

    def colmajor_bias(b, n):
        return np.ascontiguousarray(b.reshape(n, 128).T)

    bv_eff = b1r @ Wv + np.asarray(inputs["bv"], f)
    rows = np.arange(128)
    trimask = np.where(rows[:, None] > rows[None, :], -1e5, 0.0).astype(bf)

    return {
        "wq8": dr_pack(g1[:, None] * Wq, SW),
        "wk8": dr_pack(g1[:, None] * Wk, SW),
        "wv8": dr_pack(g1[:, None] * Wv, SW),
        "w1b": bf_pack(g2[:, None] * W1),
        "w28": dr_pack(W2, SW),
        "bq": colmajor_bias(b1r @ Wq + np.asarray(inputs["bq"], f), HP),
        "bk": colmajor_bias(b1r @ Wk + np.asarray(inputs["bk"], f), HP),
        "b1": colmajor_bias(b2r @ W1 + np.asarray(inputs["b1"], f), NF),
        "b2row": np.ascontiguousarray(np.asarray(inputs["b2"], f)[None, :]).astype(bf),
        "g1b": np.ascontiguousarray(np.broadcast_to(g1, (128, C))).astype(bf),
        "b1rb": np.ascontiguousarray(np.broadcast_to(b1r + bv_eff, (128, C))).astype(f),
        "trimask": np.ascontiguousarray(trimask),
        "identb": np.eye(128, dtype=f).astype(bf),
    }


def kernel(**inputs):
    from concourse.bass_utils import run_bass_kernel_spmd

    if "nc" not in _CACHE:
        _CACHE["nc"] = _build_program()
    nc = _CACHE["nc"]

    x = np.asarray(inputs["x"], np.float32)
    shared = _prep_shared(inputs)

    in_maps = []
    for c8 in range(8):
        b, c = c8 // 4, c8 % 4
        pad = 3 - c
        x_ctx = np.zeros((T, C), np.float32)
        x_ctx[pad * 128:] = x[b, 0:(13 + c) * 128]
        valid = np.zeros(NT, np.float32)
        valid[pad:] = 1.0
        m = dict(shared)
        m["x_ctx"] = x_ctx
        m["validv"] = np.ascontiguousarray(
            np.broadcast_to(valid * (1.0 / SXW), (128, NT)).astype(np.float32))
        m["vones"] = np.ascontiguousarray(
            np.broadcast_to(valid, (128, NT)).astype(np.float32))
        in_maps.append(m)

    trace = bool(int(os.environ.get("KERNEL_TRACE", "0")))
    try:
        res = run_bass_kernel_spmd(nc, in_maps, core_ids=list(range(8)), trace=trace)
    except ModuleNotFoundError:
        res = run_bass_kernel_spmd(nc, in_maps, core_ids=list(range(8)), trace=False)
    _CACHE["last_result"] = res

    out = np.empty((B, T, C), np.float32)
    for c8 in range(8):
        b, c = c8 // 4, c8 % 4
        for s in range(4):
            blk = c + 4 * s
            out[b, blk * 128:(blk + 1) * 128] = \
                res.results[c8]["out"][s * 128:(s + 1) * 128]
    return out
